# revision 22
# baseline (speedup 1.0000x reference)
"""MultiHeadLatentAttention TRN2 kernel (bf16 data path).

Sharding: 8 cores = 2 (batch) x 4 (head groups of 4 heads).
Each core computes, for its batch b and heads hg*4..hg*4+3:
  - latent down-projections kv_d, q_d (replicated within the batch group)
  - per-head up-projections K^T, Q^T (with RoPE), V
  - full attention for its 4 heads
  - partial output projection (its 512 columns of Wo's input dim)
Partial outputs are summed on the host (+ bo).

All matmul operands are bf16 (fp32 PSUM accumulation), which runs the
PE at full rate with fast weight loads and halves DMA/SBUF traffic.
Big tensors live in "feature-on-partitions" (transposed) layout so
every matmul has free dim 512.
RoPE's rotate_half is a partition-pair swap: the rope feature rows are
stored in host-permuted order (pairs (i, i+32) adjacent) so DVE
stream_shuffle(mask=i^1) implements the rotation; the sign lives in the
host-built sin table.
Softmax skips max-subtraction (scores are bounded, exp is safe);
row sums accumulate in PSUM via per-chunk ones-matmuls; reciprocals
are batched into one ACT Reciprocal at the end of the attention phase
(single table switch), then broadcast and applied to the unnormalized
attention outputs.
"""

import sys

sys.path.insert(0, "/opt/trn_rl_repo")

from contextlib import ExitStack

import numpy as np

H = 16
E = 2048
LAT = E // 4          # 512
D = E // H            # 128
R = D // 2            # 64
B, S = 2, 2048
HPC = H // 4          # 4 heads per core
NCORES = 8
NE = E // 128         # 16 contraction chunks over E
NL = LAT // 128       # 4 contraction chunks over LAT
SW = 512              # s-chunk width for projections
NSC = S // SW         # 4 s-chunks
NKC = S // 128        # 16 key chunks
QW = 1024             # q-block width in attention
NQB = S // QW         # 2 q-blocks per head
NBLK = HPC * NQB      # 8 attention blocks per core
SCALE = 1.0 / float(np.sqrt(D))
LAG = 3               # PV trails QK/exp by LAG k-chunks
DEBUG_DUMPS = False   # extra ExternalOutputs with intermediates

_RT = {}  # cached runtimes


def _mk(nc):
    """Declare DRAM I/O; returns dict of handles."""
    import concourse.mybir as mybir
    F32 = mybir.dt.float32
    BF16 = mybir.dt.bfloat16
    d = {}
    d["xT"] = nc.dram_tensor("xT", [E, S], BF16, kind="ExternalInput")
    # down-proj weights packed [m*128+p, e*128+c] (p = in-feature within
    # e-chunk on partitions, c = out-feature within m-chunk)
    d["wkvd_p"] = nc.dram_tensor("wkvd_p", [NL * 128, NE * 128], BF16,
                                 kind="ExternalInput")
    d["wqd_p"] = nc.dram_tensor("wqd_p", [NL * 128, NE * 128], BF16,
                                kind="ExternalInput")
    d["wrk_p"] = nc.dram_tensor("wrk_p", [2 * 128, NE * 128], BF16,
                                kind="ExternalInput")
    d["wkuT"] = nc.dram_tensor("wkuT", [LAT, HPC * R], BF16,
                               kind="ExternalInput")
    d["wquT"] = nc.dram_tensor("wquT", [LAT, HPC * R], BF16,
                               kind="ExternalInput")
    d["wrqT"] = nc.dram_tensor("wrqT", [LAT, HPC * R], BF16,
                               kind="ExternalInput")
    d["wvuT"] = nc.dram_tensor("wvuT", [LAT, HPC * D], BF16,
                               kind="ExternalInput")
    d["woT"] = nc.dram_tensor("woT", [HPC * D, E], BF16,
                              kind="ExternalInput")
    d["bkvd"] = nc.dram_tensor("bkvd", [128, NL], F32, kind="ExternalInput")
    d["bqd"] = nc.dram_tensor("bqd", [128, NL], F32, kind="ExternalInput")
    d["bku"] = nc.dram_tensor("bku", [128, 2], F32, kind="ExternalInput")
    d["bqu"] = nc.dram_tensor("bqu", [128, 2], F32, kind="ExternalInput")
    d["brk"] = nc.dram_tensor("brk", [128, 2], F32, kind="ExternalInput")
    d["brq"] = nc.dram_tensor("brq", [128, 2], F32, kind="ExternalInput")
    d["bvu"] = nc.dram_tensor("bvu", [1, HPC * D], F32, kind="ExternalInput")
    d["onesd"] = nc.dram_tensor("onesd", [128, 1], BF16,
                                kind="ExternalInput")
    d["cosT"] = nc.dram_tensor("cosT", [128, S], BF16, kind="ExternalInput")
    d["sinT"] = nc.dram_tensor("sinT", [128, S], BF16, kind="ExternalInput")
    d["out"] = nc.dram_tensor("out", [S, E], F32, kind="ExternalOutput")
    if DEBUG_DUMPS:
        d["dbg_sums"] = nc.dram_tensor("dbg_sums", [1, NBLK * QW], F32,
                                       kind="ExternalOutput")
        d["dbg_recip"] = nc.dram_tensor("dbg_recip", [1, NBLK * QW], BF16,
                                        kind="ExternalOutput")
        d["dbg_K"] = nc.dram_tensor("dbg_K", [128, S], BF16,
                                    kind="ExternalOutput")
        d["dbg_Q"] = nc.dram_tensor("dbg_Q", [128, S], BF16,
                                    kind="ExternalOutput")
        d["dbg_V"] = nc.dram_tensor("dbg_V", [128, HPC * D], BF16,
                                    kind="ExternalOutput")
        d["dbg_att"] = nc.dram_tensor("dbg_att", [128, S], BF16,
                                      kind="ExternalOutput")
    return d


def _consts(nc, tc, top, d):
    """Persistent tiles: K/Q/V/att storage, biases, ones, up/out weights."""
    import concourse.mybir as mybir
    F32 = mybir.dt.float32
    BF16 = mybir.dt.bfloat16

    kq_pool = top.enter_context(tc.tile_pool(name="kq", bufs=1))
    v_pool = top.enter_context(tc.tile_pool(name="vp", bufs=1))
    att_pool = top.enter_context(tc.tile_pool(name="att", bufs=1))
    cpool = top.enter_context(tc.tile_pool(name="cp", bufs=1))

    t = {}
    t["K"] = [kq_pool.tile([128, S], BF16, name=f"Kt{h}") for h in range(HPC)]
    t["Q"] = [kq_pool.tile([128, S], BF16, name=f"Qt{h}") for h in range(HPC)]
    t["V"] = [v_pool.tile([128, HPC * D], BF16, name=f"Vt{i}")
              for i in range(NKC)]
    t["att"] = [att_pool.tile([128, S], BF16, name=f"att{h}")
                for h in range(HPC)]

    def ld(name, dram, shape, dt=F32):
        tl = cpool.tile(shape, dt, name=name)
        nc.sync.dma_start(tl[:], dram[:])
        return tl

    t["ones"] = ld("ones_t", d["onesd"], [128, 1], BF16)
    t["bkvd"] = ld("bkvd_t", d["bkvd"], [128, NL])
    t["bqd"] = ld("bqd_t", d["bqd"], [128, NL])
    t["bku"] = ld("bku_t", d["bku"], [128, 2])
    t["bqu"] = ld("bqu_t", d["bqu"], [128, 2])
    t["brk"] = ld("brk_t", d["brk"], [128, 2])
    t["brq"] = ld("brq_t", d["brq"], [128, 2])

    # up-weights + rope tables: tiles allocated here, DMAs emitted inside
    # _emit_A after the first s-chunk's x/weight loads are queued, so the
    # first down-projection matmuls start as early as possible.
    upw = {}
    for nm, w in (("ku", HPC * R), ("qu", HPC * R),
                  ("rq", HPC * R), ("vu", HPC * D)):
        upw[nm] = [cpool.tile([128, w], BF16, name=f"w{nm}{l}")
                   for l in range(NL)]
    t["upw"] = upw
    t["cos"] = cpool.tile([128, S], BF16, name="cos_t")
    t["sin"] = cpool.tile([128, S], BF16, name="sin_t")
    bvu_row = ld("bvu_row", d["bvu"], [1, HPC * D])
    bvu_bc = cpool.tile([128, HPC * D], F32, name="bvu_bc")
    nc.gpsimd.partition_broadcast(bvu_bc[:], bvu_row[:])
    t["bvu_bc"] = bvu_bc

    # Wo streams in late (only needed in phase C)
    wo_t = [cpool.tile([128, E], BF16, name=f"wo{hc}") for hc in range(HPC)]
    t["wo"] = wo_t
    t["wo_dram"] = d["woT"]
    return t


def _phaseA_pools(tc, pa):
    p = {}
    p["xa"] = pa.enter_context(tc.tile_pool(name="xa", bufs=2))
    p["wa"] = pa.enter_context(tc.tile_pool(name="wa", bufs=3))
    p["kvq"] = pa.enter_context(tc.tile_pool(name="kvq", bufs=1))
    p["rp"] = pa.enter_context(tc.tile_pool(name="rp", bufs=2))
    p["psA"] = pa.enter_context(tc.tile_pool(name="psA", bufs=4,
                                             space="PSUM"))
    return p


def _emit_A(nc, tc, d, t, p):
    import concourse.mybir as mybir
    from concourse.alu_op_type import AluOpType
    F32 = mybir.dt.float32
    BF16 = mybir.dt.bfloat16
    K_t, Q_t, V_t, upw = t["K"], t["Q"], t["V"], t["upw"]
    swap_mask = [i ^ 1 for i in range(32)]

    # DMA issue spread over otherwise-idle engine queues for parallelism
    qeng = [nc.sync, nc.gpsimd, nc.scalar]

    for sc in range(NSC):
        ssl = slice(sc * SW, (sc + 1) * SW)
        xt = p["xa"].tile([128, NE * SW], BF16, name="xt")
        EPD = 2   # e-chunks per dma
        for i in range(NE // EPD):
            esl = slice(i * EPD * 128, (i + 1) * EPD * 128)
            qeng[i % 3].dma_start(
                xt[:, i * EPD * SW:(i + 1) * EPD * SW].rearrange(
                    "p (e s) -> p e s", e=EPD),
                d["xT"][esl, ssl].rearrange("(e p) s -> p e s", p=128))

        def down_mm(wdram, m):
            wt = p["wa"].tile([128, NE * 128], BF16, name="wt")
            half = NE * 128 // 2
            nc.sync.dma_start(wt[:, 0:half],
                              wdram[m * 128:(m + 1) * 128, 0:half])
            nc.gpsimd.dma_start(wt[:, half:],
                                wdram[m * 128:(m + 1) * 128, half:])
            ps = p["psA"].tile([128, SW], F32, name="psA_t")
            for e in range(NE):
                nc.tensor.matmul(ps[:], wt[:, e * 128:(e + 1) * 128],
                                 xt[:, e * SW:(e + 1) * SW],
                                 start=(e == 0), stop=(e == NE - 1))
            return ps

        def rope(ps, bias_t, m, dst):
            # ps: [128 rows = 2 heads x 64 rope rows, SW]
            xb = p["rp"].tile([128, SW], BF16, name="xb")
            nc.vector.tensor_scalar_add(xb[:], ps[:], bias_t[:, m:m + 1])
            sh = p["rp"].tile([128, SW], BF16, name="sh")
            nc.vector.stream_shuffle(sh[:], xb[:], swap_mask)
            t1 = p["rp"].tile([128, SW], BF16, name="t1")
            nc.vector.tensor_tensor(t1[:], xb[:], t["cos"][:, ssl],
                                    op=AluOpType.mult)
            t2 = p["rp"].tile([128, SW], BF16, name="t2")
            nc.vector.tensor_tensor(t2[:], sh[:], t["sin"][:, ssl],
                                    op=AluOpType.mult)
            nc.vector.tensor_tensor(dst[2 * m][R:D, ssl], t1[0:R, :],
                                    t2[0:R, :], op=AluOpType.add)
            nc.vector.tensor_tensor(dst[2 * m + 1][R:D, ssl], t1[R:D, :],
                                    t2[R:D, :], op=AluOpType.add)

        def up_mm(src, w, m):
            ps = p["psA"].tile([128, SW], F32, name="psA_t")
            for l in range(NL):
                nc.tensor.matmul(ps[:], w[l][:, m * 128:(m + 1) * 128],
                                 src[l][:], start=(l == 0),
                                 stop=(l == NL - 1))
            return ps

        # latent kv_d down-projection (replicated in batch group)
        kv_s = []
        for m in range(NL):
            ps = down_mm(d["wkvd_p"], m)
            tl = p["kvq"].tile([128, SW], BF16, name=f"lat{m}")
            nc.scalar.add(tl[:], ps[:], t["bkvd"][:, m:m + 1])
            kv_s.append(tl)
        if sc == 0:
            # stream in the up-weights + rope tables behind the first
            # down-projection's loads
            for nm, key in (("ku", "wkuT"), ("qu", "wquT"),
                            ("rq", "wrqT"), ("vu", "wvuT")):
                for l in range(NL):
                    qeng[l % 3].dma_start(
                        t["upw"][nm][l][:],
                        d[key][l * 128:(l + 1) * 128, :])
            nc.scalar.dma_start(t["cos"][:], d["cosT"][:])
            nc.scalar.dma_start(t["sin"][:], d["sinT"][:])
        for m in range(2):  # k1 -> K rows 0..63
            ps = up_mm(kv_s, upw["ku"], m)
            nc.vector.tensor_scalar_add(K_t[2 * m][0:R, ssl], ps[0:R, :],
                                        t["bku"][0:R, m:m + 1])
            nc.vector.tensor_scalar_add(K_t[2 * m + 1][0:R, ssl], ps[R:D, :],
                                        t["bku"][R:D, m:m + 1])
        for j in range(SW // 128):  # V, (s, feat) layout
            ps = p["psA"].tile([128, HPC * D], F32, name="psV_t")
            for l in range(NL):
                nc.tensor.matmul(ps[:], kv_s[l][:, j * 128:(j + 1) * 128],
                                 upw["vu"][l][:], start=(l == 0),
                                 stop=(l == NL - 1))
            nc.vector.tensor_tensor(V_t[sc * (SW // 128) + j][:], ps[:],
                                    t["bvu_bc"][:], op=AluOpType.add)

        # latent q_d down-projection
        q_s = []
        for m in range(NL):
            ps = down_mm(d["wqd_p"], m)
            tl = p["kvq"].tile([128, SW], BF16, name=f"latq{m}")
            nc.scalar.add(tl[:], ps[:], t["bqd"][:, m:m + 1])
            q_s.append(tl)
        for m in range(2):  # q1 -> Q rows 0..63
            ps = up_mm(q_s, upw["qu"], m)
            nc.vector.tensor_scalar_add(Q_t[2 * m][0:R, ssl], ps[0:R, :],
                                        t["bqu"][0:R, m:m + 1])
            nc.vector.tensor_scalar_add(Q_t[2 * m + 1][0:R, ssl], ps[R:D, :],
                                        t["bqu"][R:D, m:m + 1])
        for m in range(2):  # rope-q from q_d
            ps = up_mm(q_s, upw["rq"], m)
            rope(ps, t["brq"], m, Q_t)
        # rope-k from x
        for m in range(2):
            ps = down_mm(d["wrk_p"], m)
            rope(ps, t["brk"], m, K_t)


def _phaseB_pools(tc, pb):
    p = {}
    p["pe"] = pb.enter_context(tc.tile_pool(name="pe", bufs=6))
    p["sm"] = pb.enter_context(tc.tile_pool(name="sm", bufs=2))
    p["dr"] = pb.enter_context(tc.tile_pool(name="dr", bufs=2,
                                            space="DRAM"))
    p["cb"] = pb.enter_context(tc.tile_pool(name="cb", bufs=2))
    p["psS"] = pb.enter_context(tc.tile_pool(name="psS", bufs=2,
                                             space="PSUM"))
    p["psO"] = pb.enter_context(tc.tile_pool(name="psO", bufs=1,
                                             space="PSUM"))
    p["psR"] = pb.enter_context(tc.tile_pool(name="psR", bufs=1,
                                             space="PSUM"))
    return p


def _emit_B(nc, tc, d, t, p):
    import concourse.mybir as mybir
    from concourse.alu_op_type import AluOpType
    F32 = mybir.dt.float32
    BF16 = mybir.dt.bfloat16
    AF = mybir.ActivationFunctionType
    K_t, Q_t, V_t, att_t = t["K"], t["Q"], t["V"], t["att"]

    GRP = 4   # pe chunks pre-reduced on DVE before each sum matmul
    NCOL = QW // 128

    for h in range(HPC):
        for qp in range(NQB):
            blk = h * NQB + qp
            qa = slice(qp * QW, qp * QW + 512)
            qb = slice(qp * QW + 512, (qp + 1) * QW)
            qsl = slice(qp * QW, (qp + 1) * QW)
            oA = p["psO"].tile([128, 512], F32, name="oA")
            oB = p["psO"].tile([128, 512], F32, name="oB")
            s0 = p["psR"].tile([1, 512], F32, name="s0")
            s1 = p["psR"].tile([1, 512], F32, name="s1")
            pes = {}

            def pv(kk):
                pe = pes.pop(kk)
                nc.tensor.matmul(oA[:], V_t[kk][:, h * D:(h + 1) * D],
                                 pe[:, 0:512], start=(kk == 0),
                                 stop=(kk == NKC - 1))
                nc.tensor.matmul(oB[:], V_t[kk][:, h * D:(h + 1) * D],
                                 pe[:, 512:1024], start=(kk == 0),
                                 stop=(kk == NKC - 1))

            live = {}
            for kk in range(NKC):
                ksl = slice(kk * 128, (kk + 1) * 128)
                pp = p["psS"].tile([128, 1024], F32, name="pp")
                nc.tensor.matmul(pp[:, 0:512], K_t[h][:, ksl], Q_t[h][:, qa],
                                 start=True, stop=True)
                nc.tensor.matmul(pp[:, 512:1024], K_t[h][:, ksl],
                                 Q_t[h][:, qb], start=True, stop=True)
                pe = p["pe"].tile([128, 1024], BF16, name="pet")
                nc.scalar.activation(pe[:], pp[:], AF.Exp, scale=SCALE)
                pes[kk] = pe
                live[kk] = pe
                # 4-way DVE pre-reduction, then one accumulating ones-matmul
                if kk % GRP == GRP - 1:
                    g = kk // GRP
                    a01 = p["pe"].tile([128, 1024], BF16, name="tadd0",
                                       bufs=2)
                    a23 = p["pe"].tile([128, 1024], BF16, name="tadd1",
                                       bufs=2)
                    a03 = p["pe"].tile([128, 1024], BF16, name="tadd2",
                                       bufs=2)
                    with nc.allow_low_precision(
                            reason="4-term bf16 pre-reduction of exp "
                                   "chunks; fp32 PSUM accumulates groups"):
                        nc.vector.tensor_tensor(a01[:], live[kk - 3][:],
                                                live[kk - 2][:],
                                                op=AluOpType.add)
                        nc.vector.tensor_tensor(a23[:], live[kk - 1][:],
                                                live[kk][:],
                                                op=AluOpType.add)
                        nc.vector.tensor_tensor(a03[:], a01[:], a23[:],
                                                op=AluOpType.add)
                    live.clear()
                    nc.tensor.matmul(s0[:], t["ones"][:], a03[:, 0:512],
                                     start=(g == 0),
                                     stop=(g == NKC // GRP - 1))
                    nc.tensor.matmul(s1[:], t["ones"][:], a03[:, 512:1024],
                                     start=(g == 0),
                                     stop=(g == NKC // GRP - 1))
                if kk >= LAG:
                    pv(kk - LAG)
            for kk in range(NKC - LAG, NKC):
                pv(kk)
            # stash unnormalized attention output
            nc.vector.tensor_copy(att_t[h][:, qa], oA[:])
            nc.vector.tensor_copy(att_t[h][:, qb], oB[:])
            # per-block reciprocal pipeline: sums row -> DRAM ->
            # [128, 8] spread -> DVE reciprocal -> DRAM -> row ->
            # broadcast -> normalize.  Overlaps the next block's matmuls.
            srow = p["sm"].tile([1, QW], F32, name="srow")
            nc.scalar.copy(srow[:, 0:512], s0[:])
            nc.scalar.copy(srow[:, 512:QW], s1[:])
            dram_row = p["dr"].tile([1, QW], F32, name="dram_row")
            nc.sync.dma_start(dram_row[:], srow[:])
            rs2 = p["sm"].tile([128, NCOL], F32, name="rs2")
            nc.sync.dma_start(
                rs2[:], dram_row[:].rearrange("o (p c) -> (o p) c", p=128))
            rr2 = p["sm"].tile([128, NCOL], F32, name="rr2")
            nc.vector.reciprocal(rr2[:], rs2[:])
            rr2b = p["sm"].tile([128, NCOL], BF16, name="rr2b")
            nc.vector.tensor_copy(rr2b[:], rr2[:])
            dram_rb = p["dr"].tile([1, QW], BF16, name="dram_rb")
            nc.sync.dma_start(
                dram_rb[:].rearrange("o (p c) -> (o p) c", p=128), rr2b[:])
            recip = p["sm"].tile([1, QW], BF16, name="recip")
            nc.sync.dma_start(recip[:], dram_rb[:])
            cb = p["cb"].tile([128, QW], BF16, name="cbt")
            nc.gpsimd.partition_broadcast(cb[:], recip[:])
            nc.vector.tensor_tensor(att_t[h][:, qsl], att_t[h][:, qsl],
                                    cb[:], op=AluOpType.mult)
            if DEBUG_DUMPS and blk < 2:
                nc.sync.dma_start(
                    d["dbg_sums"][:, blk * QW:(blk + 1) * QW], srow[:])
                nc.sync.dma_start(
                    d["dbg_recip"][:, blk * QW:(blk + 1) * QW], recip[:])
    if DEBUG_DUMPS:
        nc.sync.dma_start(d["dbg_K"][:], t["K"][0][:])
        nc.sync.dma_start(d["dbg_Q"][:], t["Q"][0][:])
        nc.sync.dma_start(d["dbg_V"][:], t["V"][0][:])
        nc.sync.dma_start(d["dbg_att"][:], t["att"][0][:])


def _phaseC_pools(tc, pc):
    p = {}
    p["oc"] = pc.enter_context(tc.tile_pool(name="oc", bufs=4))
    p["psC"] = pc.enter_context(tc.tile_pool(name="psC", bufs=8,
                                             space="PSUM"))
    return p


def _emit_C(nc, tc, d, t, p):
    import concourse.mybir as mybir
    F32 = mybir.dt.float32
    att_t, wo_t = t["att"], t["wo"]

    qeng = [nc.sync, nc.gpsimd, nc.scalar]
    for hc in range(HPC):
        qeng[hc % 3].dma_start(wo_t[hc][:],
                               t["wo_dram"][hc * 128:(hc + 1) * 128, :])

    for sj in range(S // 128):
        pss = [p["psC"].tile([128, 512], F32, name="psC_t")
               for _ in range(E // 512)]
        for hc in range(HPC):
            for ocn in range(E // 512):
                nc.tensor.matmul(pss[ocn][:],
                                 att_t[hc][:, sj * 128:(sj + 1) * 128],
                                 wo_t[hc][:, ocn * 512:(ocn + 1) * 512],
                                 start=(hc == 0), stop=(hc == HPC - 1))
        for ocn in range(E // 512):
            ob = p["oc"].tile([128, 512], F32, name="ob")
            if ocn % 2 == 0:
                nc.vector.tensor_copy(ob[:], pss[ocn][:])
            else:
                nc.scalar.copy(ob[:], pss[ocn][:])
            qeng[(sj * 4 + ocn) % 3].dma_start(
                d["out"][sj * 128:(sj + 1) * 128,
                         ocn * 512:(ocn + 1) * 512], ob[:])


def _build_program():
    import concourse.bacc as bacc
    import concourse.tile as tile

    nc = bacc.Bacc("TRN2", target_bir_lowering=False, debug=False,
                   num_devices=NCORES)
    d = _mk(nc)

    with tile.TileContext(nc) as tc, ExitStack() as top:
        t = _consts(nc, tc, top, d)
        with ExitStack() as pa:
            pA = _phaseA_pools(tc, pa)
            _emit_A(nc, tc, d, t, pA)
        with ExitStack() as pb:
            pB = _phaseB_pools(tc, pb)
            _emit_B(nc, tc, d, t, pB)
        with ExitStack() as pc:
            pC = _phaseC_pools(tc, pc)
            _emit_C(nc, tc, d, t, pC)

    nc.compile()
    return nc


def _rope_tables():
    inv_freq = 1.0 / (10000.0 ** (np.arange(0, R, 2, dtype=np.float64) / R))
    t = np.arange(S, dtype=np.float64)
    freqs = np.outer(t, inv_freq)                       # (S, R/2)
    emb = np.concatenate([freqs, freqs], axis=-1)       # (S, R)
    cos = np.cos(emb).astype(np.float32)                # (S, R)
    sin = np.sin(emb).astype(np.float32)
    perm = np.array([(j // 2) if j % 2 == 0 else (j // 2) + R // 2
                     for j in range(R)])
    sign = np.array([-1.0 if j % 2 == 0 else 1.0
                     for j in range(R)], dtype=np.float32)
    cos_p = cos[:, perm].T.copy()                       # (R, S)
    sin_p = (sin[:, perm] * sign[None, :]).T.copy()     # (R, S)
    cosT = np.concatenate([cos_p, cos_p], axis=0)       # (128, S)
    sinT = np.concatenate([sin_p, sin_p], axis=0)
    return cosT, sinT, perm


def _pack_down(Wm, nm):
    """Pack [nm*128, E] weight into [m*128+p, e*128+c] layout (bf16)."""
    import ml_dtypes
    a = Wm.reshape(nm, 128, NE, 128)        # [m, c, e, p]
    a = a.transpose(0, 3, 2, 1)             # [m, p, e, c]
    return np.ascontiguousarray(
        a.reshape(nm * 128, NE * 128).astype(ml_dtypes.bfloat16))


def _per_core_inputs(inputs, core):
    import ml_dtypes
    bf = ml_dtypes.bfloat16
    b, hg = divmod(core, HPC)
    cosT, sinT, perm = _rope_tables()
    hsl64 = np.concatenate([hg * HPC * R + h * R + perm
                            for h in range(HPC)])       # permuted rope rows
    hs64 = slice(hg * HPC * R, (hg + 1) * HPC * R)      # natural 64-rows
    hs128 = slice(hg * HPC * D, (hg + 1) * HPC * D)     # natural 128-rows

    x = np.asarray(inputs["x"], dtype=np.float32)
    f = np.float32
    im = {
        "xT": np.ascontiguousarray(x[b].T.astype(bf)),
        "wkvd_p": _pack_down(np.asarray(inputs["Wkv_d"], f), NL),
        "wqd_p": _pack_down(np.asarray(inputs["Wq_d"], f), NL),
        "wrk_p": _pack_down(np.asarray(inputs["Wrk"], f)[hsl64], 2),
        "wkuT": np.ascontiguousarray(
            np.asarray(inputs["Wk_u"], f)[hs64].T.astype(bf)),
        "wquT": np.ascontiguousarray(
            np.asarray(inputs["Wq_u"], f)[hs64].T.astype(bf)),
        "wrqT": np.ascontiguousarray(
            np.asarray(inputs["Wrq"], f)[hsl64].T.astype(bf)),
        "wvuT": np.ascontiguousarray(
            np.asarray(inputs["Wv_u"], f)[hs128].T.astype(bf)),
        "woT": np.ascontiguousarray(
            np.asarray(inputs["Wo"], f).T[hs128].astype(bf)),
        "bkvd": np.ascontiguousarray(
            np.asarray(inputs["bkv_d"], f).reshape(NL, 128).T),
        "bqd": np.ascontiguousarray(
            np.asarray(inputs["bq_d"], f).reshape(NL, 128).T),
        "bku": np.ascontiguousarray(
            np.asarray(inputs["bk_u"], f)[hs64].reshape(2, 128).T),
        "bqu": np.ascontiguousarray(
            np.asarray(inputs["bq_u"], f)[hs64].reshape(2, 128).T),
        "brk": np.ascontiguousarray(
            np.asarray(inputs["brk"], f)[hsl64].reshape(2, 128).T),
        "brq": np.ascontiguousarray(
            np.asarray(inputs["brq"], f)[hsl64].reshape(2, 128).T),
        "bvu": np.ascontiguousarray(
            np.asarray(inputs["bv_u"], f)[hs128].reshape(1, HPC * D)),
        "onesd": np.ones((128, 1), dtype=bf),
        "cosT": cosT.astype(bf),
        "sinT": sinT.astype(bf),
    }
    return im


def _get_runtime():
    if "rt" in _RT:
        return _RT["rt"]
    import jax
    import numpy as _np
    from jax.sharding import Mesh, PartitionSpec
    from jax.experimental.shard_map import shard_map

    import concourse.mybir as mybir
    from concourse import bass2jax

    nc = _build_program()
    bass2jax.install_neuronx_cc_hook()

    partition_name = (nc.partition_id_tensor.name
                      if nc.partition_id_tensor else None)
    in_names, out_names, out_avals, zero_shapes = [], [], [], []
    for alloc in nc.m.functions[0].allocations:
        if not isinstance(alloc, mybir.MemoryLocationSet):
            continue
        name = alloc.memorylocations[0].name
        if alloc.kind == "ExternalInput":
            if name != partition_name:
                in_names.append(name)
        elif alloc.kind == "ExternalOutput":
            out_names.append(name)
            np_dt = mybir.dt.np(alloc.dtype)
            out_avals.append(jax.core.ShapedArray(
                tuple(alloc.tensor_shape), np_dt))
            zero_shapes.append((tuple(alloc.tensor_shape), np_dt))

    n_params = len(in_names)
    n_outs = len(out_names)
    all_in_names = list(in_names) + list(out_names)
    if partition_name is not None:
        all_in_names.append(partition_name)

    def _body(*args):
        operands = list(args)
        if partition_name is not None:
            operands.append(bass2jax.partition_id_tensor())
        outs = bass2jax._bass_exec_p.bind(
            *operands,
            out_avals=tuple(out_avals),
            in_names=tuple(all_in_names),
            out_names=tuple(out_names),
            lowering_input_output_aliases=(),
            sim_require_finite=True,
            sim_require_nnan=True,
            nc=nc,
        )
        return tuple(outs)

    devices = jax.devices()[:NCORES]
    mesh = Mesh(_np.asarray(devices), ("core",))
    in_specs = (PartitionSpec("core"),) * (n_params + n_outs)
    out_specs = (PartitionSpec("core"),) * n_outs
    donate = tuple(range(n_params, n_params + n_outs))
    sharded = jax.jit(
        shard_map(_body, mesh=mesh, in_specs=in_specs, out_specs=out_specs,
                  check_rep=False),
        donate_argnums=donate, keep_unused=True)

    _RT["rt"] = dict(nc=nc, sharded=sharded, in_names=in_names,
                     out_names=out_names, zero_shapes=zero_shapes,
                     n_outs=n_outs)
    return _RT["rt"]


def _run_cores(in_maps):
    rt = _get_runtime()
    import numpy as _np
    concat_in = [
        _np.concatenate([in_maps[c][name] for c in range(NCORES)], axis=0)
        for name in rt["in_names"]
    ]
    concat_zeros = [
        _np.zeros((NCORES * shp[0],) + shp[1:], dt)
        for (shp, dt) in rt["zero_shapes"]
    ]
    out_arrs = rt["sharded"](*concat_in, *concat_zeros)
    res = []
    for c in range(NCORES):
        m = {}
        for i, name in enumerate(rt["out_names"]):
            shp, dt = rt["zero_shapes"][i]
            m[name] = _np.asarray(out_arrs[i]).reshape((NCORES,) + shp)[c]
        res.append(m)
    return res


def kernel(**inputs):
    in_maps = [_per_core_inputs(inputs, c) for c in range(NCORES)]
    res = _run_cores(in_maps)
    bo = np.asarray(inputs["bo"], dtype=np.float32)
    final = np.empty((B, S, E), dtype=np.float32)
    for b in range(B):
        acc = res[HPC * b]["out"].astype(np.float32).copy()
        for g in range(1, HPC):
            acc += res[HPC * b + g]["out"]
        final[b] = acc + bo[None, :]
    return final


# revision 24
# speedup vs baseline: 1.0011x; 1.0011x over previous
"""MultiHeadLatentAttention TRN2 kernel (bf16 data path).

Sharding: 8 cores = 2 (batch) x 4 (head groups of 4 heads).
Each core computes, for its batch b and heads hg*4..hg*4+3:
  - latent down-projections kv_d, q_d (replicated within the batch group)
  - per-head up-projections K^T, Q^T (with RoPE), V
  - full attention for its 4 heads
  - partial output projection (its 512 columns of Wo's input dim)
Partial outputs are summed on the host (+ bo).

All matmul operands are bf16 (fp32 PSUM accumulation), which runs the
PE at full rate with fast weight loads and halves DMA/SBUF traffic.
Big tensors live in "feature-on-partitions" (transposed) layout so
every matmul has free dim 512.
RoPE's rotate_half is a partition-pair swap: the rope feature rows are
stored in host-permuted order (pairs (i, i+32) adjacent) so DVE
stream_shuffle(mask=i^1) implements the rotation; the sign lives in the
host-built sin table.
Softmax skips max-subtraction (scores are bounded, exp is safe);
row sums accumulate in PSUM via per-chunk ones-matmuls; reciprocals
are batched into one ACT Reciprocal at the end of the attention phase
(single table switch), then broadcast and applied to the unnormalized
attention outputs.
"""

import sys

sys.path.insert(0, "/opt/trn_rl_repo")

from contextlib import ExitStack

import numpy as np

H = 16
E = 2048
LAT = E // 4          # 512
D = E // H            # 128
R = D // 2            # 64
B, S = 2, 2048
HPC = H // 4          # 4 heads per core
NCORES = 8
NE = E // 128         # 16 contraction chunks over E
NL = LAT // 128       # 4 contraction chunks over LAT
SW = 512              # s-chunk width for projections
NSC = S // SW         # 4 s-chunks
NKC = S // 128        # 16 key chunks
QW = 1024             # q-block width in attention
NQB = S // QW         # 2 q-blocks per head
NBLK = HPC * NQB      # 8 attention blocks per core
SCALE = 1.0 / float(np.sqrt(D))
LAG = 3               # PV trails QK/exp by LAG k-chunks
DEBUG_DUMPS = False   # extra ExternalOutputs with intermediates

_RT = {}  # cached runtimes


def _mk(nc):
    """Declare DRAM I/O; returns dict of handles."""
    import concourse.mybir as mybir
    F32 = mybir.dt.float32
    BF16 = mybir.dt.bfloat16
    d = {}
    d["xT"] = nc.dram_tensor("xT", [E, S], BF16, kind="ExternalInput")
    # down-proj weights packed [m*128+p, e*128+c] (p = in-feature within
    # e-chunk on partitions, c = out-feature within m-chunk)
    d["wkvd_p"] = nc.dram_tensor("wkvd_p", [NL * 128, NE * 128], BF16,
                                 kind="ExternalInput")
    d["wqd_p"] = nc.dram_tensor("wqd_p", [NL * 128, NE * 128], BF16,
                                kind="ExternalInput")
    d["wrk_p"] = nc.dram_tensor("wrk_p", [2 * 128, NE * 128], BF16,
                                kind="ExternalInput")
    d["wkuT"] = nc.dram_tensor("wkuT", [LAT, HPC * R], BF16,
                               kind="ExternalInput")
    d["wquT"] = nc.dram_tensor("wquT", [LAT, HPC * R], BF16,
                               kind="ExternalInput")
    d["wrqT"] = nc.dram_tensor("wrqT", [LAT, HPC * R], BF16,
                               kind="ExternalInput")
    d["wvuT"] = nc.dram_tensor("wvuT", [LAT, HPC * D], BF16,
                               kind="ExternalInput")
    d["woT"] = nc.dram_tensor("woT", [HPC * D, E], BF16,
                              kind="ExternalInput")
    d["bkvd"] = nc.dram_tensor("bkvd", [128, NL], F32, kind="ExternalInput")
    d["bqd"] = nc.dram_tensor("bqd", [128, NL], F32, kind="ExternalInput")
    d["bku"] = nc.dram_tensor("bku", [128, 2], F32, kind="ExternalInput")
    d["bqu"] = nc.dram_tensor("bqu", [128, 2], F32, kind="ExternalInput")
    d["brk"] = nc.dram_tensor("brk", [128, 2], F32, kind="ExternalInput")
    d["brq"] = nc.dram_tensor("brq", [128, 2], F32, kind="ExternalInput")
    d["bvu"] = nc.dram_tensor("bvu", [1, HPC * D], F32, kind="ExternalInput")
    d["onesd"] = nc.dram_tensor("onesd", [128, 1], BF16,
                                kind="ExternalInput")
    d["cosT"] = nc.dram_tensor("cosT", [128, S], BF16, kind="ExternalInput")
    d["sinT"] = nc.dram_tensor("sinT", [128, S], BF16, kind="ExternalInput")
    d["out"] = nc.dram_tensor("out", [S, E], F32, kind="ExternalOutput")
    if DEBUG_DUMPS:
        d["dbg_sums"] = nc.dram_tensor("dbg_sums", [1, NBLK * QW], F32,
                                       kind="ExternalOutput")
        d["dbg_recip"] = nc.dram_tensor("dbg_recip", [1, NBLK * QW], BF16,
                                        kind="ExternalOutput")
        d["dbg_K"] = nc.dram_tensor("dbg_K", [128, S], BF16,
                                    kind="ExternalOutput")
        d["dbg_Q"] = nc.dram_tensor("dbg_Q", [128, S], BF16,
                                    kind="ExternalOutput")
        d["dbg_V"] = nc.dram_tensor("dbg_V", [128, HPC * D], BF16,
                                    kind="ExternalOutput")
        d["dbg_att"] = nc.dram_tensor("dbg_att", [128, S], BF16,
                                      kind="ExternalOutput")
    return d


def _consts(nc, tc, top, d):
    """Persistent tiles: K/Q/V/att storage, biases, ones, up/out weights."""
    import concourse.mybir as mybir
    F32 = mybir.dt.float32
    BF16 = mybir.dt.bfloat16

    kq_pool = top.enter_context(tc.tile_pool(name="kq", bufs=1))
    v_pool = top.enter_context(tc.tile_pool(name="vp", bufs=1))
    att_pool = top.enter_context(tc.tile_pool(name="att", bufs=1))
    cpool = top.enter_context(tc.tile_pool(name="cp", bufs=1))

    t = {}
    t["K"] = [kq_pool.tile([128, S], BF16, name=f"Kt{h}") for h in range(HPC)]
    t["Q"] = [kq_pool.tile([128, S], BF16, name=f"Qt{h}") for h in range(HPC)]
    t["V"] = [v_pool.tile([128, HPC * D], BF16, name=f"Vt{i}")
              for i in range(NKC)]
    t["att"] = [att_pool.tile([128, S], BF16, name=f"att{h}")
                for h in range(HPC)]

    def ld(name, dram, shape, dt=F32):
        tl = cpool.tile(shape, dt, name=name)
        nc.sync.dma_start(tl[:], dram[:])
        return tl

    t["ones"] = ld("ones_t", d["onesd"], [128, 1], BF16)
    t["bkvd"] = ld("bkvd_t", d["bkvd"], [128, NL])
    t["bqd"] = ld("bqd_t", d["bqd"], [128, NL])
    t["bku"] = ld("bku_t", d["bku"], [128, 2])
    t["bqu"] = ld("bqu_t", d["bqu"], [128, 2])
    t["brk"] = ld("brk_t", d["brk"], [128, 2])
    t["brq"] = ld("brq_t", d["brq"], [128, 2])

    # up-weights + rope tables: tiles allocated here, DMAs emitted inside
    # _emit_A after the first s-chunk's x/weight loads are queued, so the
    # first down-projection matmuls start as early as possible.
    upw = {}
    for nm, w in (("ku", HPC * R), ("qu", HPC * R),
                  ("rq", HPC * R), ("vu", HPC * D)):
        upw[nm] = [cpool.tile([128, w], BF16, name=f"w{nm}{l}")
                   for l in range(NL)]
    t["upw"] = upw
    t["cos"] = cpool.tile([128, S], BF16, name="cos_t")
    t["sin"] = cpool.tile([128, S], BF16, name="sin_t")
    bvu_row = ld("bvu_row", d["bvu"], [1, HPC * D])
    bvu_bc = cpool.tile([128, HPC * D], F32, name="bvu_bc")
    nc.gpsimd.partition_broadcast(bvu_bc[:], bvu_row[:])
    t["bvu_bc"] = bvu_bc

    # Wo streams in late (only needed in phase C)
    wo_t = [cpool.tile([128, E], BF16, name=f"wo{hc}") for hc in range(HPC)]
    t["wo"] = wo_t
    t["wo_dram"] = d["woT"]
    return t


def _phaseA_pools(tc, pa):
    p = {}
    p["xa"] = pa.enter_context(tc.tile_pool(name="xa", bufs=2))
    p["wa"] = pa.enter_context(tc.tile_pool(name="wa", bufs=3))
    p["kvq"] = pa.enter_context(tc.tile_pool(name="kvq", bufs=1))
    p["rp"] = pa.enter_context(tc.tile_pool(name="rp", bufs=2))
    p["psA"] = pa.enter_context(tc.tile_pool(name="psA", bufs=4,
                                             space="PSUM"))
    return p


def _emit_A(nc, tc, d, t, p):
    import concourse.mybir as mybir
    from concourse.alu_op_type import AluOpType
    F32 = mybir.dt.float32
    BF16 = mybir.dt.bfloat16
    K_t, Q_t, V_t, upw = t["K"], t["Q"], t["V"], t["upw"]
    swap_mask = [i ^ 1 for i in range(32)]

    # DMA issue spread over otherwise-idle engine queues for parallelism
    qeng = [nc.sync, nc.gpsimd, nc.scalar]

    for sc in range(NSC):
        ssl = slice(sc * SW, (sc + 1) * SW)
        EPD = 2   # e-chunks per tile/dma
        xts = []
        for i in range(NE // EPD):
            esl = slice(i * EPD * 128, (i + 1) * EPD * 128)
            xi = p["xa"].tile([128, EPD * SW], BF16, name=f"xt{i}")
            qeng[i % 3].dma_start(
                xi[:].rearrange("p (e s) -> p e s", e=EPD),
                d["xT"][esl, ssl].rearrange("(e p) s -> p e s", p=128))
            xts.append(xi)

        def xchunk(e):
            return xts[e // EPD][:, (e % EPD) * SW:(e % EPD + 1) * SW]

        def down_mm(wdram, m):
            wt = p["wa"].tile([128, NE * 128], BF16, name="wt")
            half = NE * 128 // 2
            nc.sync.dma_start(wt[:, 0:half],
                              wdram[m * 128:(m + 1) * 128, 0:half])
            nc.gpsimd.dma_start(wt[:, half:],
                                wdram[m * 128:(m + 1) * 128, half:])
            ps = p["psA"].tile([128, SW], F32, name="psA_t")
            for e in range(NE):
                nc.tensor.matmul(ps[:], wt[:, e * 128:(e + 1) * 128],
                                 xchunk(e),
                                 start=(e == 0), stop=(e == NE - 1))
            return ps

        def rope(ps, bias_t, m, dst):
            # ps: [128 rows = 2 heads x 64 rope rows, SW]
            xb = p["rp"].tile([128, SW], BF16, name="xb")
            nc.vector.tensor_scalar_add(xb[:], ps[:], bias_t[:, m:m + 1])
            sh = p["rp"].tile([128, SW], BF16, name="sh")
            nc.vector.stream_shuffle(sh[:], xb[:], swap_mask)
            t1 = p["rp"].tile([128, SW], BF16, name="t1")
            nc.vector.tensor_tensor(t1[:], xb[:], t["cos"][:, ssl],
                                    op=AluOpType.mult)
            t2 = p["rp"].tile([128, SW], BF16, name="t2")
            nc.vector.tensor_tensor(t2[:], sh[:], t["sin"][:, ssl],
                                    op=AluOpType.mult)
            nc.vector.tensor_tensor(dst[2 * m][R:D, ssl], t1[0:R, :],
                                    t2[0:R, :], op=AluOpType.add)
            nc.vector.tensor_tensor(dst[2 * m + 1][R:D, ssl], t1[R:D, :],
                                    t2[R:D, :], op=AluOpType.add)

        def up_mm(src, w, m):
            ps = p["psA"].tile([128, SW], F32, name="psA_t")
            for l in range(NL):
                nc.tensor.matmul(ps[:], w[l][:, m * 128:(m + 1) * 128],
                                 src[l][:], start=(l == 0),
                                 stop=(l == NL - 1))
            return ps

        # latent kv_d down-projection (replicated in batch group)
        kv_s = []
        for m in range(NL):
            ps = down_mm(d["wkvd_p"], m)
            tl = p["kvq"].tile([128, SW], BF16, name=f"lat{m}")
            nc.scalar.add(tl[:], ps[:], t["bkvd"][:, m:m + 1])
            kv_s.append(tl)
        if sc == 0:
            # stream in the up-weights + rope tables behind the first
            # down-projection's loads
            for nm, key in (("ku", "wkuT"), ("qu", "wquT"),
                            ("rq", "wrqT"), ("vu", "wvuT")):
                for l in range(NL):
                    qeng[l % 3].dma_start(
                        t["upw"][nm][l][:],
                        d[key][l * 128:(l + 1) * 128, :])
            nc.scalar.dma_start(t["cos"][:], d["cosT"][:])
            nc.scalar.dma_start(t["sin"][:], d["sinT"][:])
        for m in range(2):  # k1 -> K rows 0..63
            ps = up_mm(kv_s, upw["ku"], m)
            nc.vector.tensor_scalar_add(K_t[2 * m][0:R, ssl], ps[0:R, :],
                                        t["bku"][0:R, m:m + 1])
            nc.vector.tensor_scalar_add(K_t[2 * m + 1][0:R, ssl], ps[R:D, :],
                                        t["bku"][R:D, m:m + 1])
        for j in range(SW // 128):  # V, (s, feat) layout
            ps = p["psA"].tile([128, HPC * D], F32, name="psV_t")
            for l in range(NL):
                nc.tensor.matmul(ps[:], kv_s[l][:, j * 128:(j + 1) * 128],
                                 upw["vu"][l][:], start=(l == 0),
                                 stop=(l == NL - 1))
            nc.vector.tensor_tensor(V_t[sc * (SW // 128) + j][:], ps[:],
                                    t["bvu_bc"][:], op=AluOpType.add)

        # latent q_d down-projection
        q_s = []
        for m in range(NL):
            ps = down_mm(d["wqd_p"], m)
            tl = p["kvq"].tile([128, SW], BF16, name=f"latq{m}")
            nc.scalar.add(tl[:], ps[:], t["bqd"][:, m:m + 1])
            q_s.append(tl)
        for m in range(2):  # q1 -> Q rows 0..63
            ps = up_mm(q_s, upw["qu"], m)
            nc.vector.tensor_scalar_add(Q_t[2 * m][0:R, ssl], ps[0:R, :],
                                        t["bqu"][0:R, m:m + 1])
            nc.vector.tensor_scalar_add(Q_t[2 * m + 1][0:R, ssl], ps[R:D, :],
                                        t["bqu"][R:D, m:m + 1])
        for m in range(2):  # rope-q from q_d
            ps = up_mm(q_s, upw["rq"], m)
            rope(ps, t["brq"], m, Q_t)
        # rope-k from x
        for m in range(2):
            ps = down_mm(d["wrk_p"], m)
            rope(ps, t["brk"], m, K_t)


def _phaseB_pools(tc, pb):
    p = {}
    p["pe"] = pb.enter_context(tc.tile_pool(name="pe", bufs=6))
    p["sm"] = pb.enter_context(tc.tile_pool(name="sm", bufs=2))
    p["dr"] = pb.enter_context(tc.tile_pool(name="dr", bufs=2,
                                            space="DRAM"))
    p["cb"] = pb.enter_context(tc.tile_pool(name="cb", bufs=2))
    p["psS"] = pb.enter_context(tc.tile_pool(name="psS", bufs=2,
                                             space="PSUM"))
    p["psO"] = pb.enter_context(tc.tile_pool(name="psO", bufs=1,
                                             space="PSUM"))
    p["psR"] = pb.enter_context(tc.tile_pool(name="psR", bufs=1,
                                             space="PSUM"))
    return p


def _emit_B(nc, tc, d, t, p):
    import concourse.mybir as mybir
    from concourse.alu_op_type import AluOpType
    F32 = mybir.dt.float32
    BF16 = mybir.dt.bfloat16
    AF = mybir.ActivationFunctionType
    K_t, Q_t, V_t, att_t = t["K"], t["Q"], t["V"], t["att"]

    GRP = 4   # pe chunks pre-reduced on DVE before each sum matmul
    NCOL = QW // 128

    for h in range(HPC):
        for qp in range(NQB):
            blk = h * NQB + qp
            qa = slice(qp * QW, qp * QW + 512)
            qb = slice(qp * QW + 512, (qp + 1) * QW)
            qsl = slice(qp * QW, (qp + 1) * QW)
            oA = p["psO"].tile([128, 512], F32, name="oA")
            oB = p["psO"].tile([128, 512], F32, name="oB")
            s0 = p["psR"].tile([1, 512], F32, name="s0")
            s1 = p["psR"].tile([1, 512], F32, name="s1")
            pes = {}

            def pv(kk):
                pe = pes.pop(kk)
                nc.tensor.matmul(oA[:], V_t[kk][:, h * D:(h + 1) * D],
                                 pe[:, 0:512], start=(kk == 0),
                                 stop=(kk == NKC - 1))
                nc.tensor.matmul(oB[:], V_t[kk][:, h * D:(h + 1) * D],
                                 pe[:, 512:1024], start=(kk == 0),
                                 stop=(kk == NKC - 1))

            live = {}
            for kk in range(NKC):
                ksl = slice(kk * 128, (kk + 1) * 128)
                pp = p["psS"].tile([128, 1024], F32, name="pp")
                nc.tensor.matmul(pp[:, 0:512], K_t[h][:, ksl], Q_t[h][:, qa],
                                 start=True, stop=True)
                nc.tensor.matmul(pp[:, 512:1024], K_t[h][:, ksl],
                                 Q_t[h][:, qb], start=True, stop=True)
                pe = p["pe"].tile([128, 1024], BF16, name="pet")
                nc.scalar.activation(pe[:], pp[:], AF.Exp, scale=SCALE)
                pes[kk] = pe
                live[kk] = pe
                # 4-way DVE pre-reduction, then one accumulating ones-matmul
                if kk % GRP == GRP - 1:
                    g = kk // GRP
                    a01 = p["pe"].tile([128, 1024], BF16, name="tadd0",
                                       bufs=2)
                    a23 = p["pe"].tile([128, 1024], BF16, name="tadd1",
                                       bufs=2)
                    a03 = p["pe"].tile([128, 1024], BF16, name="tadd2",
                                       bufs=2)
                    with nc.allow_low_precision(
                            reason="4-term bf16 pre-reduction of exp "
                                   "chunks; fp32 PSUM accumulates groups"):
                        nc.vector.tensor_tensor(a01[:], live[kk - 3][:],
                                                live[kk - 2][:],
                                                op=AluOpType.add)
                        nc.vector.tensor_tensor(a23[:], live[kk - 1][:],
                                                live[kk][:],
                                                op=AluOpType.add)
                        nc.vector.tensor_tensor(a03[:], a01[:], a23[:],
                                                op=AluOpType.add)
                    live.clear()
                    nc.tensor.matmul(s0[:], t["ones"][:], a03[:, 0:512],
                                     start=(g == 0),
                                     stop=(g == NKC // GRP - 1))
                    nc.tensor.matmul(s1[:], t["ones"][:], a03[:, 512:1024],
                                     start=(g == 0),
                                     stop=(g == NKC // GRP - 1))
                if kk >= LAG:
                    pv(kk - LAG)
            for kk in range(NKC - LAG, NKC):
                pv(kk)
            # stash unnormalized attention output
            nc.vector.tensor_copy(att_t[h][:, qa], oA[:])
            nc.vector.tensor_copy(att_t[h][:, qb], oB[:])
            # per-block reciprocal pipeline: sums row -> DRAM ->
            # [128, 8] spread -> DVE reciprocal -> DRAM -> row ->
            # broadcast -> normalize.  Overlaps the next block's matmuls.
            srow = p["sm"].tile([1, QW], F32, name="srow")
            nc.scalar.copy(srow[:, 0:512], s0[:])
            nc.scalar.copy(srow[:, 512:QW], s1[:])
            dram_row = p["dr"].tile([1, QW], F32, name="dram_row")
            nc.sync.dma_start(dram_row[:], srow[:])
            rs2 = p["sm"].tile([128, NCOL], F32, name="rs2")
            nc.sync.dma_start(
                rs2[:], dram_row[:].rearrange("o (p c) -> (o p) c", p=128))
            rr2 = p["sm"].tile([128, NCOL], F32, name="rr2")
            nc.vector.reciprocal(rr2[:], rs2[:])
            rr2b = p["sm"].tile([128, NCOL], BF16, name="rr2b")
            nc.vector.tensor_copy(rr2b[:], rr2[:])
            dram_rb = p["dr"].tile([1, QW], BF16, name="dram_rb")
            nc.sync.dma_start(
                dram_rb[:].rearrange("o (p c) -> (o p) c", p=128), rr2b[:])
            recip = p["sm"].tile([1, QW], BF16, name="recip")
            nc.sync.dma_start(recip[:], dram_rb[:])
            cb = p["cb"].tile([128, QW], BF16, name="cbt")
            nc.gpsimd.partition_broadcast(cb[:], recip[:])
            nc.vector.tensor_tensor(att_t[h][:, qsl], att_t[h][:, qsl],
                                    cb[:], op=AluOpType.mult)
            if DEBUG_DUMPS and blk < 2:
                nc.sync.dma_start(
                    d["dbg_sums"][:, blk * QW:(blk + 1) * QW], srow[:])
                nc.sync.dma_start(
                    d["dbg_recip"][:, blk * QW:(blk + 1) * QW], recip[:])
    if DEBUG_DUMPS:
        nc.sync.dma_start(d["dbg_K"][:], t["K"][0][:])
        nc.sync.dma_start(d["dbg_Q"][:], t["Q"][0][:])
        nc.sync.dma_start(d["dbg_V"][:], t["V"][0][:])
        nc.sync.dma_start(d["dbg_att"][:], t["att"][0][:])


def _phaseC_pools(tc, pc):
    p = {}
    p["oc"] = pc.enter_context(tc.tile_pool(name="oc", bufs=4))
    p["psC"] = pc.enter_context(tc.tile_pool(name="psC", bufs=8,
                                             space="PSUM"))
    return p


def _emit_C(nc, tc, d, t, p):
    import concourse.mybir as mybir
    F32 = mybir.dt.float32
    att_t, wo_t = t["att"], t["wo"]

    qeng = [nc.sync, nc.gpsimd, nc.scalar]
    for hc in range(HPC):
        qeng[hc % 3].dma_start(wo_t[hc][:],
                               t["wo_dram"][hc * 128:(hc + 1) * 128, :])

    for sj in range(S // 128):
        pss = [p["psC"].tile([128, 512], F32, name="psC_t")
               for _ in range(E // 512)]
        for hc in range(HPC):
            for ocn in range(E // 512):
                nc.tensor.matmul(pss[ocn][:],
                                 att_t[hc][:, sj * 128:(sj + 1) * 128],
                                 wo_t[hc][:, ocn * 512:(ocn + 1) * 512],
                                 start=(hc == 0), stop=(hc == HPC - 1))
        for ocn in range(E // 512):
            ob = p["oc"].tile([128, 512], F32, name="ob")
            if ocn % 2 == 0:
                nc.vector.tensor_copy(ob[:], pss[ocn][:])
            else:
                nc.scalar.copy(ob[:], pss[ocn][:])
            qeng[(sj * 4 + ocn) % 3].dma_start(
                d["out"][sj * 128:(sj + 1) * 128,
                         ocn * 512:(ocn + 1) * 512], ob[:])


def _build_program():
    import concourse.bacc as bacc
    import concourse.tile as tile

    nc = bacc.Bacc("TRN2", target_bir_lowering=False, debug=False,
                   num_devices=NCORES)
    d = _mk(nc)

    with tile.TileContext(nc) as tc, ExitStack() as top:
        t = _consts(nc, tc, top, d)
        with ExitStack() as pa:
            pA = _phaseA_pools(tc, pa)
            _emit_A(nc, tc, d, t, pA)
        with ExitStack() as pb:
            pB = _phaseB_pools(tc, pb)
            _emit_B(nc, tc, d, t, pB)
        with ExitStack() as pc:
            pC = _phaseC_pools(tc, pc)
            _emit_C(nc, tc, d, t, pC)

    nc.compile()
    return nc


def _rope_tables():
    inv_freq = 1.0 / (10000.0 ** (np.arange(0, R, 2, dtype=np.float64) / R))
    t = np.arange(S, dtype=np.float64)
    freqs = np.outer(t, inv_freq)                       # (S, R/2)
    emb = np.concatenate([freqs, freqs], axis=-1)       # (S, R)
    cos = np.cos(emb).astype(np.float32)                # (S, R)
    sin = np.sin(emb).astype(np.float32)
    perm = np.array([(j // 2) if j % 2 == 0 else (j // 2) + R // 2
                     for j in range(R)])
    sign = np.array([-1.0 if j % 2 == 0 else 1.0
                     for j in range(R)], dtype=np.float32)
    cos_p = cos[:, perm].T.copy()                       # (R, S)
    sin_p = (sin[:, perm] * sign[None, :]).T.copy()     # (R, S)
    cosT = np.concatenate([cos_p, cos_p], axis=0)       # (128, S)
    sinT = np.concatenate([sin_p, sin_p], axis=0)
    return cosT, sinT, perm


def _pack_down(Wm, nm):
    """Pack [nm*128, E] weight into [m*128+p, e*128+c] layout (bf16)."""
    import ml_dtypes
    a = Wm.reshape(nm, 128, NE, 128)        # [m, c, e, p]
    a = a.transpose(0, 3, 2, 1)             # [m, p, e, c]
    return np.ascontiguousarray(
        a.reshape(nm * 128, NE * 128).astype(ml_dtypes.bfloat16))


def _per_core_inputs(inputs, core):
    import ml_dtypes
    bf = ml_dtypes.bfloat16
    b, hg = divmod(core, HPC)
    cosT, sinT, perm = _rope_tables()
    hsl64 = np.concatenate([hg * HPC * R + h * R + perm
                            for h in range(HPC)])       # permuted rope rows
    hs64 = slice(hg * HPC * R, (hg + 1) * HPC * R)      # natural 64-rows
    hs128 = slice(hg * HPC * D, (hg + 1) * HPC * D)     # natural 128-rows

    x = np.asarray(inputs["x"], dtype=np.float32)
    f = np.float32
    im = {
        "xT": np.ascontiguousarray(x[b].T.astype(bf)),
        "wkvd_p": _pack_down(np.asarray(inputs["Wkv_d"], f), NL),
        "wqd_p": _pack_down(np.asarray(inputs["Wq_d"], f), NL),
        "wrk_p": _pack_down(np.asarray(inputs["Wrk"], f)[hsl64], 2),
        "wkuT": np.ascontiguousarray(
            np.asarray(inputs["Wk_u"], f)[hs64].T.astype(bf)),
        "wquT": np.ascontiguousarray(
            np.asarray(inputs["Wq_u"], f)[hs64].T.astype(bf)),
        "wrqT": np.ascontiguousarray(
            np.asarray(inputs["Wrq"], f)[hsl64].T.astype(bf)),
        "wvuT": np.ascontiguousarray(
            np.asarray(inputs["Wv_u"], f)[hs128].T.astype(bf)),
        "woT": np.ascontiguousarray(
            np.asarray(inputs["Wo"], f).T[hs128].astype(bf)),
        "bkvd": np.ascontiguousarray(
            np.asarray(inputs["bkv_d"], f).reshape(NL, 128).T),
        "bqd": np.ascontiguousarray(
            np.asarray(inputs["bq_d"], f).reshape(NL, 128).T),
        "bku": np.ascontiguousarray(
            np.asarray(inputs["bk_u"], f)[hs64].reshape(2, 128).T),
        "bqu": np.ascontiguousarray(
            np.asarray(inputs["bq_u"], f)[hs64].reshape(2, 128).T),
        "brk": np.ascontiguousarray(
            np.asarray(inputs["brk"], f)[hsl64].reshape(2, 128).T),
        "brq": np.ascontiguousarray(
            np.asarray(inputs["brq"], f)[hsl64].reshape(2, 128).T),
        "bvu": np.ascontiguousarray(
            np.asarray(inputs["bv_u"], f)[hs128].reshape(1, HPC * D)),
        "onesd": np.ones((128, 1), dtype=bf),
        "cosT": cosT.astype(bf),
        "sinT": sinT.astype(bf),
    }
    return im


def _get_runtime():
    if "rt" in _RT:
        return _RT["rt"]
    import jax
    import numpy as _np
    from jax.sharding import Mesh, PartitionSpec
    from jax.experimental.shard_map import shard_map

    import concourse.mybir as mybir
    from concourse import bass2jax

    nc = _build_program()
    bass2jax.install_neuronx_cc_hook()

    partition_name = (nc.partition_id_tensor.name
                      if nc.partition_id_tensor else None)
    in_names, out_names, out_avals, zero_shapes = [], [], [], []
    for alloc in nc.m.functions[0].allocations:
        if not isinstance(alloc, mybir.MemoryLocationSet):
            continue
        name = alloc.memorylocations[0].name
        if alloc.kind == "ExternalInput":
            if name != partition_name:
                in_names.append(name)
        elif alloc.kind == "ExternalOutput":
            out_names.append(name)
            np_dt = mybir.dt.np(alloc.dtype)
            out_avals.append(jax.core.ShapedArray(
                tuple(alloc.tensor_shape), np_dt))
            zero_shapes.append((tuple(alloc.tensor_shape), np_dt))

    n_params = len(in_names)
    n_outs = len(out_names)
    all_in_names = list(in_names) + list(out_names)
    if partition_name is not None:
        all_in_names.append(partition_name)

    def _body(*args):
        operands = list(args)
        if partition_name is not None:
            operands.append(bass2jax.partition_id_tensor())
        outs = bass2jax._bass_exec_p.bind(
            *operands,
            out_avals=tuple(out_avals),
            in_names=tuple(all_in_names),
            out_names=tuple(out_names),
            lowering_input_output_aliases=(),
            sim_require_finite=True,
            sim_require_nnan=True,
            nc=nc,
        )
        return tuple(outs)

    devices = jax.devices()[:NCORES]
    mesh = Mesh(_np.asarray(devices), ("core",))
    in_specs = (PartitionSpec("core"),) * (n_params + n_outs)
    out_specs = (PartitionSpec("core"),) * n_outs
    donate = tuple(range(n_params, n_params + n_outs))
    sharded = jax.jit(
        shard_map(_body, mesh=mesh, in_specs=in_specs, out_specs=out_specs,
                  check_rep=False),
        donate_argnums=donate, keep_unused=True)

    _RT["rt"] = dict(nc=nc, sharded=sharded, in_names=in_names,
                     out_names=out_names, zero_shapes=zero_shapes,
                     n_outs=n_outs)
    return _RT["rt"]


def _run_cores(in_maps):
    rt = _get_runtime()
    import numpy as _np
    concat_in = [
        _np.concatenate([in_maps[c][name] for c in range(NCORES)], axis=0)
        for name in rt["in_names"]
    ]
    concat_zeros = [
        _np.zeros((NCORES * shp[0],) + shp[1:], dt)
        for (shp, dt) in rt["zero_shapes"]
    ]
    out_arrs = rt["sharded"](*concat_in, *concat_zeros)
    res = []
    for c in range(NCORES):
        m = {}
        for i, name in enumerate(rt["out_names"]):
            shp, dt = rt["zero_shapes"][i]
            m[name] = _np.asarray(out_arrs[i]).reshape((NCORES,) + shp)[c]
        res.append(m)
    return res


def kernel(**inputs):
    in_maps = [_per_core_inputs(inputs, c) for c in range(NCORES)]
    res = _run_cores(in_maps)
    bo = np.asarray(inputs["bo"], dtype=np.float32)
    final = np.empty((B, S, E), dtype=np.float32)
    for b in range(B):
        acc = res[HPC * b]["out"].astype(np.float32).copy()
        for g in range(1, HPC):
            acc += res[HPC * b + g]["out"]
        final[b] = acc + bo[None, :]
    return final


# revision 30
# speedup vs baseline: 1.0531x; 1.0520x over previous
"""MultiHeadLatentAttention TRN2 kernel (bf16 data path).

Sharding: 8 cores = 2 (batch) x 4 (head groups of 4 heads).
Each core computes, for its batch b and heads hg*4..hg*4+3:
  - latent down-projections kv_d, q_d (replicated within the batch group)
  - per-head up-projections K^T, Q^T (with RoPE), V
  - full attention for its 4 heads
  - partial output projection (its 512 columns of Wo's input dim)
Partial outputs are summed on the host (+ bo).

All matmul operands are bf16 (fp32 PSUM accumulation), which runs the
PE at full rate with fast weight loads and halves DMA/SBUF traffic.
Big tensors live in "feature-on-partitions" (transposed) layout so
every matmul has free dim 512.
RoPE's rotate_half is a partition-pair swap: the rope feature rows are
stored in host-permuted order (pairs (i, i+32) adjacent) so DVE
stream_shuffle(mask=i^1) implements the rotation; the sign lives in the
host-built sin table.
Softmax skips max-subtraction (scores are bounded, exp is safe);
row sums accumulate in PSUM via per-chunk ones-matmuls; reciprocals
are batched into one ACT Reciprocal at the end of the attention phase
(single table switch), then broadcast and applied to the unnormalized
attention outputs.
"""

import sys

sys.path.insert(0, "/opt/trn_rl_repo")

from contextlib import ExitStack

import numpy as np

H = 16
E = 2048
LAT = E // 4          # 512
D = E // H            # 128
R = D // 2            # 64
B, S = 2, 2048
HPC = H // 4          # 4 heads per core
NCORES = 8
NE = E // 128         # 16 contraction chunks over E
NL = LAT // 128       # 4 contraction chunks over LAT
SW = 512              # s-chunk width for projections
NSC = S // SW         # 4 s-chunks
NKC = S // 128        # 16 key chunks
QW = 1024             # q-block width in attention
NQB = S // QW         # 2 q-blocks per head
NBLK = HPC * NQB      # 8 attention blocks per core
SCALE = 1.0 / float(np.sqrt(D))
LAG = 3               # PV trails QK/exp by LAG k-chunks
DEBUG_DUMPS = False   # extra ExternalOutputs with intermediates

_RT = {}  # cached runtimes


def _mk(nc):
    """Declare DRAM I/O; returns dict of handles."""
    import concourse.mybir as mybir
    F32 = mybir.dt.float32
    BF16 = mybir.dt.bfloat16
    d = {}
    F8 = mybir.dt.float8e4
    d["xT"] = nc.dram_tensor("xT", [E, S], BF16, kind="ExternalInput")
    # fp8 copy of x, pre-scaled by 1/8 (weights carry the 8x) so both
    # operands sit in e4m3's normal range; used by the logits-only
    # q_d / rope-k down-projections
    d["xT8"] = nc.dram_tensor("xT8", [E, S], F8, kind="ExternalInput")
    # down-proj weights packed [m*128+p, e*128+c] (p = in-feature within
    # e-chunk on partitions, c = out-feature within m-chunk)
    d["wkvd_p"] = nc.dram_tensor("wkvd_p", [NL * 128, NE * 128], BF16,
                                 kind="ExternalInput")
    d["wqd_p8"] = nc.dram_tensor("wqd_p8", [NL * 128, NE * 128], F8,
                                 kind="ExternalInput")
    d["wrk_p8"] = nc.dram_tensor("wrk_p8", [2 * 128, NE * 128], F8,
                                 kind="ExternalInput")
    d["wkuT"] = nc.dram_tensor("wkuT", [LAT, HPC * R], BF16,
                               kind="ExternalInput")
    d["wquT"] = nc.dram_tensor("wquT", [LAT, HPC * R], BF16,
                               kind="ExternalInput")
    d["wrqT"] = nc.dram_tensor("wrqT", [LAT, HPC * R], BF16,
                               kind="ExternalInput")
    d["wvuT"] = nc.dram_tensor("wvuT", [LAT, HPC * D], BF16,
                               kind="ExternalInput")
    d["woT"] = nc.dram_tensor("woT", [HPC * D, E], BF16,
                              kind="ExternalInput")
    d["bkvd"] = nc.dram_tensor("bkvd", [128, NL], F32, kind="ExternalInput")
    d["bqd"] = nc.dram_tensor("bqd", [128, NL], F32, kind="ExternalInput")
    d["bku"] = nc.dram_tensor("bku", [128, 2], F32, kind="ExternalInput")
    d["bqu"] = nc.dram_tensor("bqu", [128, 2], F32, kind="ExternalInput")
    d["brk"] = nc.dram_tensor("brk", [128, 2], F32, kind="ExternalInput")
    d["brq"] = nc.dram_tensor("brq", [128, 2], F32, kind="ExternalInput")
    d["bvu"] = nc.dram_tensor("bvu", [1, HPC * D], F32, kind="ExternalInput")
    d["onesd"] = nc.dram_tensor("onesd", [128, 1], BF16,
                                kind="ExternalInput")
    d["cosT"] = nc.dram_tensor("cosT", [128, S], BF16, kind="ExternalInput")
    d["sinT"] = nc.dram_tensor("sinT", [128, S], BF16, kind="ExternalInput")
    d["out"] = nc.dram_tensor("out", [S, E], F32, kind="ExternalOutput")
    if DEBUG_DUMPS:
        d["dbg_sums"] = nc.dram_tensor("dbg_sums", [1, NBLK * QW], F32,
                                       kind="ExternalOutput")
        d["dbg_recip"] = nc.dram_tensor("dbg_recip", [1, NBLK * QW], BF16,
                                        kind="ExternalOutput")
        d["dbg_K"] = nc.dram_tensor("dbg_K", [128, S], BF16,
                                    kind="ExternalOutput")
        d["dbg_Q"] = nc.dram_tensor("dbg_Q", [128, S], BF16,
                                    kind="ExternalOutput")
        d["dbg_V"] = nc.dram_tensor("dbg_V", [128, HPC * D], BF16,
                                    kind="ExternalOutput")
        d["dbg_att"] = nc.dram_tensor("dbg_att", [128, S], BF16,
                                      kind="ExternalOutput")
    return d


def _consts(nc, tc, top, d):
    """Persistent tiles: K/Q/V/att storage, biases, ones, up/out weights."""
    import concourse.mybir as mybir
    F32 = mybir.dt.float32
    BF16 = mybir.dt.bfloat16

    kq_pool = top.enter_context(tc.tile_pool(name="kq", bufs=1))
    v_pool = top.enter_context(tc.tile_pool(name="vp", bufs=1))
    att_pool = top.enter_context(tc.tile_pool(name="att", bufs=1))
    cpool = top.enter_context(tc.tile_pool(name="cp", bufs=1))

    t = {}
    t["K"] = [kq_pool.tile([128, S], BF16, name=f"Kt{h}") for h in range(HPC)]
    t["Q"] = [kq_pool.tile([128, S], BF16, name=f"Qt{h}") for h in range(HPC)]
    t["V"] = [v_pool.tile([128, HPC * D], BF16, name=f"Vt{i}")
              for i in range(NKC)]
    t["att"] = [att_pool.tile([128, S], BF16, name=f"att{h}")
                for h in range(HPC)]

    def ld(name, dram, shape, dt=F32):
        tl = cpool.tile(shape, dt, name=name)
        nc.sync.dma_start(tl[:], dram[:])
        return tl

    t["ones"] = ld("ones_t", d["onesd"], [128, 1], BF16)
    t["bkvd"] = ld("bkvd_t", d["bkvd"], [128, NL])
    t["bqd"] = ld("bqd_t", d["bqd"], [128, NL])
    t["bku"] = ld("bku_t", d["bku"], [128, 2])
    t["bqu"] = ld("bqu_t", d["bqu"], [128, 2])
    t["brk"] = ld("brk_t", d["brk"], [128, 2])
    t["brq"] = ld("brq_t", d["brq"], [128, 2])

    # up-weights + rope tables: tiles allocated here, DMAs emitted inside
    # _emit_A after the first s-chunk's x/weight loads are queued, so the
    # first down-projection matmuls start as early as possible.
    upw = {}
    for nm, w in (("ku", HPC * R), ("qu", HPC * R),
                  ("rq", HPC * R), ("vu", HPC * D)):
        upw[nm] = [cpool.tile([128, w], BF16, name=f"w{nm}{l}")
                   for l in range(NL)]
    t["upw"] = upw
    t["cos"] = cpool.tile([128, S], BF16, name="cos_t")
    t["sin"] = cpool.tile([128, S], BF16, name="sin_t")
    bvu_row = ld("bvu_row", d["bvu"], [1, HPC * D])
    bvu_bc = cpool.tile([128, HPC * D], F32, name="bvu_bc")
    nc.gpsimd.partition_broadcast(bvu_bc[:], bvu_row[:])
    t["bvu_bc"] = bvu_bc

    # Wo streams in late (only needed in phase C)
    wo_t = [cpool.tile([128, E], BF16, name=f"wo{hc}") for hc in range(HPC)]
    t["wo"] = wo_t
    t["wo_dram"] = d["woT"]
    return t


def _phaseA_pools(tc, pa):
    p = {}
    p["xa"] = pa.enter_context(tc.tile_pool(name="xa", bufs=2))
    p["wa"] = pa.enter_context(tc.tile_pool(name="wa", bufs=3))
    p["kvq"] = pa.enter_context(tc.tile_pool(name="kvq", bufs=1))
    p["rp"] = pa.enter_context(tc.tile_pool(name="rp", bufs=2))
    p["psA"] = pa.enter_context(tc.tile_pool(name="psA", bufs=4,
                                             space="PSUM"))
    return p


def _emit_A(nc, tc, d, t, p):
    import concourse.mybir as mybir
    from concourse.alu_op_type import AluOpType
    F32 = mybir.dt.float32
    BF16 = mybir.dt.bfloat16
    F8 = mybir.dt.float8e4
    DR = mybir.MatmulPerfMode.DoubleRow
    K_t, Q_t, V_t, upw = t["K"], t["Q"], t["V"], t["upw"]
    swap_mask = [i ^ 1 for i in range(32)]

    # DMA issue spread over otherwise-idle engine queues for parallelism
    qeng = [nc.sync, nc.gpsimd, nc.scalar]

    for sc in range(NSC):
        ssl = slice(sc * SW, (sc + 1) * SW)
        EPD = 2   # e-chunks per tile/dma
        xts = []
        xts8 = []
        for i in range(NE // EPD):
            esl = slice(i * EPD * 128, (i + 1) * EPD * 128)
            xi = p["xa"].tile([128, EPD * SW], BF16, name=f"xt{i}")
            qeng[i % 3].dma_start(
                xi[:].rearrange("p (e s) -> p e s", e=EPD),
                d["xT"][esl, ssl].rearrange("(e p) s -> p e s", p=128))
            xts.append(xi)
            x8 = p["xa"].tile([128, EPD * SW], F8, name=f"xt8_{i}")
            qeng[(i + 1) % 3].dma_start(
                x8[:].rearrange("p (e s) -> p e s", e=EPD),
                d["xT8"][esl, ssl].rearrange("(e p) s -> p e s", p=128))
            xts8.append(x8)

        def xchunk(e):
            return xts[e // EPD][:, (e % EPD) * SW:(e % EPD + 1) * SW]

        def down_mm(wdram, m):
            wt = p["wa"].tile([128, NE * 128], BF16, name="wt")
            half = NE * 128 // 2
            nc.sync.dma_start(wt[:, 0:half],
                              wdram[m * 128:(m + 1) * 128, 0:half])
            nc.gpsimd.dma_start(wt[:, half:],
                                wdram[m * 128:(m + 1) * 128, half:])
            ps = p["psA"].tile([128, SW], F32, name="psA_t")
            for e in range(NE):
                nc.tensor.matmul(ps[:], wt[:, e * 128:(e + 1) * 128],
                                 xchunk(e),
                                 start=(e == 0), stop=(e == NE - 1))
            return ps

        def down_mm8(wdram, m):
            # fp8 DoubleRow: each matmul contracts an e-chunk PAIR via
            # 3D APs [p, 2, f]
            wt8 = p["wa"].tile([128, NE * 128], F8, name="wt8")
            half = NE * 128 // 2
            nc.sync.dma_start(wt8[:, 0:half],
                              wdram[m * 128:(m + 1) * 128, 0:half])
            nc.gpsimd.dma_start(wt8[:, half:],
                                wdram[m * 128:(m + 1) * 128, half:])
            w3 = wt8[:].rearrange("p (e c) -> p e c", e=NE)
            ps = p["psA"].tile([128, SW], F32, name="psA_t")
            for i in range(NE // 2):
                nc.tensor.matmul(
                    ps[:], w3[:, 2 * i:2 * i + 2, :],
                    xts8[i][:].rearrange("p (e s) -> p e s", e=2),
                    start=(i == 0), stop=(i == NE // 2 - 1),
                    perf_mode=DR)
            return ps

        def rope(ps, bias_t, m, dst):
            # ps: [128 rows = 2 heads x 64 rope rows, SW]
            xb = p["rp"].tile([128, SW], BF16, name="xb")
            nc.vector.tensor_scalar_add(xb[:], ps[:], bias_t[:, m:m + 1])
            sh = p["rp"].tile([128, SW], BF16, name="sh")
            nc.vector.stream_shuffle(sh[:], xb[:], swap_mask)
            t1 = p["rp"].tile([128, SW], BF16, name="t1")
            nc.vector.tensor_tensor(t1[:], xb[:], t["cos"][:, ssl],
                                    op=AluOpType.mult)
            t2 = p["rp"].tile([128, SW], BF16, name="t2")
            nc.vector.tensor_tensor(t2[:], sh[:], t["sin"][:, ssl],
                                    op=AluOpType.mult)
            nc.vector.tensor_tensor(dst[2 * m][R:D, ssl], t1[0:R, :],
                                    t2[0:R, :], op=AluOpType.add)
            nc.vector.tensor_tensor(dst[2 * m + 1][R:D, ssl], t1[R:D, :],
                                    t2[R:D, :], op=AluOpType.add)

        def up_mm(src, w, m):
            ps = p["psA"].tile([128, SW], F32, name="psA_t")
            for l in range(NL):
                nc.tensor.matmul(ps[:], w[l][:, m * 128:(m + 1) * 128],
                                 src[l][:], start=(l == 0),
                                 stop=(l == NL - 1))
            return ps

        # latent kv_d down-projection (replicated in batch group)
        kv_s = []
        for m in range(NL):
            ps = down_mm(d["wkvd_p"], m)
            tl = p["kvq"].tile([128, SW], BF16, name=f"lat{m}")
            nc.scalar.add(tl[:], ps[:], t["bkvd"][:, m:m + 1])
            kv_s.append(tl)
        if sc == 0:
            # stream in the up-weights + rope tables behind the first
            # down-projection's loads
            for nm, key in (("ku", "wkuT"), ("qu", "wquT"),
                            ("rq", "wrqT"), ("vu", "wvuT")):
                for l in range(NL):
                    qeng[l % 3].dma_start(
                        t["upw"][nm][l][:],
                        d[key][l * 128:(l + 1) * 128, :])
            nc.scalar.dma_start(t["cos"][:], d["cosT"][:])
            nc.scalar.dma_start(t["sin"][:], d["sinT"][:])
        for m in range(2):  # k1 -> K rows 0..63
            ps = up_mm(kv_s, upw["ku"], m)
            nc.vector.tensor_scalar_add(K_t[2 * m][0:R, ssl], ps[0:R, :],
                                        t["bku"][0:R, m:m + 1])
            nc.vector.tensor_scalar_add(K_t[2 * m + 1][0:R, ssl], ps[R:D, :],
                                        t["bku"][R:D, m:m + 1])
        for j in range(SW // 128):  # V, (s, feat) layout
            ps = p["psA"].tile([128, HPC * D], F32, name="psV_t")
            for l in range(NL):
                nc.tensor.matmul(ps[:], kv_s[l][:, j * 128:(j + 1) * 128],
                                 upw["vu"][l][:], start=(l == 0),
                                 stop=(l == NL - 1))
            nc.vector.tensor_tensor(V_t[sc * (SW // 128) + j][:], ps[:],
                                    t["bvu_bc"][:], op=AluOpType.add)

        # latent q_d down-projection (fp8 DoubleRow, logits-only path)
        q_s = []
        for m in range(NL):
            ps = down_mm8(d["wqd_p8"], m)
            tl = p["kvq"].tile([128, SW], BF16, name=f"latq{m}")
            nc.scalar.add(tl[:], ps[:], t["bqd"][:, m:m + 1])
            q_s.append(tl)
        for m in range(2):  # q1 -> Q rows 0..63
            ps = up_mm(q_s, upw["qu"], m)
            nc.vector.tensor_scalar_add(Q_t[2 * m][0:R, ssl], ps[0:R, :],
                                        t["bqu"][0:R, m:m + 1])
            nc.vector.tensor_scalar_add(Q_t[2 * m + 1][0:R, ssl], ps[R:D, :],
                                        t["bqu"][R:D, m:m + 1])
        for m in range(2):  # rope-q from q_d
            ps = up_mm(q_s, upw["rq"], m)
            rope(ps, t["brq"], m, Q_t)
        # rope-k from x (fp8 DoubleRow, logits-only path)
        for m in range(2):
            ps = down_mm8(d["wrk_p8"], m)
            rope(ps, t["brk"], m, K_t)


def _phaseB_pools(tc, pb):
    p = {}
    p["pe"] = pb.enter_context(tc.tile_pool(name="pe", bufs=6))
    p["sm"] = pb.enter_context(tc.tile_pool(name="sm", bufs=2))
    p["dr"] = pb.enter_context(tc.tile_pool(name="dr", bufs=2,
                                            space="DRAM"))
    p["cb"] = pb.enter_context(tc.tile_pool(name="cb", bufs=2))
    p["psS"] = pb.enter_context(tc.tile_pool(name="psS", bufs=2,
                                             space="PSUM"))
    p["psO"] = pb.enter_context(tc.tile_pool(name="psO", bufs=1,
                                             space="PSUM"))
    p["psR"] = pb.enter_context(tc.tile_pool(name="psR", bufs=1,
                                             space="PSUM"))
    return p


def _emit_B(nc, tc, d, t, p):
    import concourse.mybir as mybir
    from concourse.alu_op_type import AluOpType
    F32 = mybir.dt.float32
    BF16 = mybir.dt.bfloat16
    AF = mybir.ActivationFunctionType
    K_t, Q_t, V_t, att_t = t["K"], t["Q"], t["V"], t["att"]

    GRP = 4   # pe chunks pre-reduced on DVE before each sum matmul
    NCOL = QW // 128

    for h in range(HPC):
        for qp in range(NQB):
            blk = h * NQB + qp
            qa = slice(qp * QW, qp * QW + 512)
            qb = slice(qp * QW + 512, (qp + 1) * QW)
            qsl = slice(qp * QW, (qp + 1) * QW)
            oA = p["psO"].tile([128, 512], F32, name="oA")
            oB = p["psO"].tile([128, 512], F32, name="oB")
            s0 = p["psR"].tile([1, 512], F32, name="s0")
            s1 = p["psR"].tile([1, 512], F32, name="s1")
            pes = {}

            def pv(kk):
                pe = pes.pop(kk)
                nc.tensor.matmul(oA[:], V_t[kk][:, h * D:(h + 1) * D],
                                 pe[:, 0:512], start=(kk == 0),
                                 stop=(kk == NKC - 1))
                nc.tensor.matmul(oB[:], V_t[kk][:, h * D:(h + 1) * D],
                                 pe[:, 512:1024], start=(kk == 0),
                                 stop=(kk == NKC - 1))

            live = {}
            for kk in range(NKC):
                ksl = slice(kk * 128, (kk + 1) * 128)
                pp = p["psS"].tile([128, 1024], F32, name="pp")
                nc.tensor.matmul(pp[:, 0:512], K_t[h][:, ksl], Q_t[h][:, qa],
                                 start=True, stop=True)
                nc.tensor.matmul(pp[:, 512:1024], K_t[h][:, ksl],
                                 Q_t[h][:, qb], start=True, stop=True)
                pe = p["pe"].tile([128, 1024], BF16, name="pet")
                nc.scalar.activation(pe[:], pp[:], AF.Exp, scale=SCALE)
                pes[kk] = pe
                live[kk] = pe
                # 4-way DVE pre-reduction, then one accumulating ones-matmul
                if kk % GRP == GRP - 1:
                    g = kk // GRP
                    a01 = p["pe"].tile([128, 1024], BF16, name="tadd0",
                                       bufs=2)
                    a23 = p["pe"].tile([128, 1024], BF16, name="tadd1",
                                       bufs=2)
                    a03 = p["pe"].tile([128, 1024], BF16, name="tadd2",
                                       bufs=2)
                    with nc.allow_low_precision(
                            reason="4-term bf16 pre-reduction of exp "
                                   "chunks; fp32 PSUM accumulates groups"):
                        nc.vector.tensor_tensor(a01[:], live[kk - 3][:],
                                                live[kk - 2][:],
                                                op=AluOpType.add)
                        nc.vector.tensor_tensor(a23[:], live[kk - 1][:],
                                                live[kk][:],
                                                op=AluOpType.add)
                        nc.vector.tensor_tensor(a03[:], a01[:], a23[:],
                                                op=AluOpType.add)
                    live.clear()
                    nc.tensor.matmul(s0[:], t["ones"][:], a03[:, 0:512],
                                     start=(g == 0),
                                     stop=(g == NKC // GRP - 1))
                    nc.tensor.matmul(s1[:], t["ones"][:], a03[:, 512:1024],
                                     start=(g == 0),
                                     stop=(g == NKC // GRP - 1))
                if kk >= LAG:
                    pv(kk - LAG)
            for kk in range(NKC - LAG, NKC):
                pv(kk)
            # stash unnormalized attention output
            nc.vector.tensor_copy(att_t[h][:, qa], oA[:])
            nc.vector.tensor_copy(att_t[h][:, qb], oB[:])
            # per-block reciprocal pipeline: sums row -> DRAM ->
            # [128, 8] spread -> DVE reciprocal -> DRAM -> row ->
            # broadcast -> normalize.  Overlaps the next block's matmuls.
            srow = p["sm"].tile([1, QW], F32, name="srow")
            nc.scalar.copy(srow[:, 0:512], s0[:])
            nc.scalar.copy(srow[:, 512:QW], s1[:])
            dram_row = p["dr"].tile([1, QW], F32, name="dram_row")
            nc.sync.dma_start(dram_row[:], srow[:])
            rs2 = p["sm"].tile([128, NCOL], F32, name="rs2")
            nc.sync.dma_start(
                rs2[:], dram_row[:].rearrange("o (p c) -> (o p) c", p=128))
            rr2 = p["sm"].tile([128, NCOL], F32, name="rr2")
            nc.vector.reciprocal(rr2[:], rs2[:])
            rr2b = p["sm"].tile([128, NCOL], BF16, name="rr2b")
            nc.vector.tensor_copy(rr2b[:], rr2[:])
            dram_rb = p["dr"].tile([1, QW], BF16, name="dram_rb")
            nc.sync.dma_start(
                dram_rb[:].rearrange("o (p c) -> (o p) c", p=128), rr2b[:])
            recip = p["sm"].tile([1, QW], BF16, name="recip")
            nc.sync.dma_start(recip[:], dram_rb[:])
            cb = p["cb"].tile([128, QW], BF16, name="cbt")
            nc.gpsimd.partition_broadcast(cb[:], recip[:])
            nc.vector.tensor_tensor(att_t[h][:, qsl], att_t[h][:, qsl],
                                    cb[:], op=AluOpType.mult)
            if DEBUG_DUMPS and blk < 2:
                nc.sync.dma_start(
                    d["dbg_sums"][:, blk * QW:(blk + 1) * QW], srow[:])
                nc.sync.dma_start(
                    d["dbg_recip"][:, blk * QW:(blk + 1) * QW], recip[:])
    if DEBUG_DUMPS:
        nc.sync.dma_start(d["dbg_K"][:], t["K"][0][:])
        nc.sync.dma_start(d["dbg_Q"][:], t["Q"][0][:])
        nc.sync.dma_start(d["dbg_V"][:], t["V"][0][:])
        nc.sync.dma_start(d["dbg_att"][:], t["att"][0][:])


def _phaseC_pools(tc, pc):
    p = {}
    p["oc"] = pc.enter_context(tc.tile_pool(name="oc", bufs=4))
    p["psC"] = pc.enter_context(tc.tile_pool(name="psC", bufs=8,
                                             space="PSUM"))
    return p


def _emit_C(nc, tc, d, t, p):
    import concourse.mybir as mybir
    F32 = mybir.dt.float32
    att_t, wo_t = t["att"], t["wo"]

    qeng = [nc.sync, nc.gpsimd, nc.scalar]
    for hc in range(HPC):
        qeng[hc % 3].dma_start(wo_t[hc][:],
                               t["wo_dram"][hc * 128:(hc + 1) * 128, :])

    for sj in range(S // 128):
        pss = [p["psC"].tile([128, 512], F32, name="psC_t")
               for _ in range(E // 512)]
        for hc in range(HPC):
            for ocn in range(E // 512):
                nc.tensor.matmul(pss[ocn][:],
                                 att_t[hc][:, sj * 128:(sj + 1) * 128],
                                 wo_t[hc][:, ocn * 512:(ocn + 1) * 512],
                                 start=(hc == 0), stop=(hc == HPC - 1))
        for ocn in range(E // 512):
            ob = p["oc"].tile([128, 512], F32, name="ob")
            if ocn % 2 == 0:
                nc.vector.tensor_copy(ob[:], pss[ocn][:])
            else:
                nc.scalar.copy(ob[:], pss[ocn][:])
            qeng[(sj * 4 + ocn) % 3].dma_start(
                d["out"][sj * 128:(sj + 1) * 128,
                         ocn * 512:(ocn + 1) * 512], ob[:])


def _build_program():
    import concourse.bacc as bacc
    import concourse.tile as tile

    nc = bacc.Bacc("TRN2", target_bir_lowering=False, debug=False,
                   num_devices=NCORES)
    d = _mk(nc)

    with tile.TileContext(nc) as tc, ExitStack() as top:
        t = _consts(nc, tc, top, d)
        with ExitStack() as pa:
            pA = _phaseA_pools(tc, pa)
            _emit_A(nc, tc, d, t, pA)
        with ExitStack() as pb:
            pB = _phaseB_pools(tc, pb)
            _emit_B(nc, tc, d, t, pB)
        with ExitStack() as pc:
            pC = _phaseC_pools(tc, pc)
            _emit_C(nc, tc, d, t, pC)

    nc.compile()
    return nc


def _rope_tables():
    inv_freq = 1.0 / (10000.0 ** (np.arange(0, R, 2, dtype=np.float64) / R))
    t = np.arange(S, dtype=np.float64)
    freqs = np.outer(t, inv_freq)                       # (S, R/2)
    emb = np.concatenate([freqs, freqs], axis=-1)       # (S, R)
    cos = np.cos(emb).astype(np.float32)                # (S, R)
    sin = np.sin(emb).astype(np.float32)
    perm = np.array([(j // 2) if j % 2 == 0 else (j // 2) + R // 2
                     for j in range(R)])
    sign = np.array([-1.0 if j % 2 == 0 else 1.0
                     for j in range(R)], dtype=np.float32)
    cos_p = cos[:, perm].T.copy()                       # (R, S)
    sin_p = (sin[:, perm] * sign[None, :]).T.copy()     # (R, S)
    cosT = np.concatenate([cos_p, cos_p], axis=0)       # (128, S)
    sinT = np.concatenate([sin_p, sin_p], axis=0)
    return cosT, sinT, perm


def _pack_down(Wm, nm, dt=None):
    """Pack [nm*128, E] weight into [m*128+p, e*128+c] layout."""
    import ml_dtypes
    if dt is None:
        dt = ml_dtypes.bfloat16
    a = Wm.reshape(nm, 128, NE, 128)        # [m, c, e, p]
    a = a.transpose(0, 3, 2, 1)             # [m, p, e, c]
    return np.ascontiguousarray(
        a.reshape(nm * 128, NE * 128).astype(dt))


def _per_core_inputs(inputs, core):
    import ml_dtypes
    bf = ml_dtypes.bfloat16
    b, hg = divmod(core, HPC)
    cosT, sinT, perm = _rope_tables()
    hsl64 = np.concatenate([hg * HPC * R + h * R + perm
                            for h in range(HPC)])       # permuted rope rows
    hs64 = slice(hg * HPC * R, (hg + 1) * HPC * R)      # natural 64-rows
    hs128 = slice(hg * HPC * D, (hg + 1) * HPC * D)     # natural 128-rows

    x = np.asarray(inputs["x"], dtype=np.float32)
    f = np.float32
    e4 = ml_dtypes.float8_e4m3
    im = {
        "xT": np.ascontiguousarray(x[b].T.astype(bf)),
        "xT8": np.ascontiguousarray((x[b].T / 8.0).astype(e4)),
        "wkvd_p": _pack_down(np.asarray(inputs["Wkv_d"], f), NL),
        "wqd_p8": _pack_down(8.0 * np.asarray(inputs["Wq_d"], f), NL, e4),
        "wrk_p8": _pack_down(8.0 * np.asarray(inputs["Wrk"], f)[hsl64], 2,
                             e4),
        "wkuT": np.ascontiguousarray(
            np.asarray(inputs["Wk_u"], f)[hs64].T.astype(bf)),
        "wquT": np.ascontiguousarray(
            np.asarray(inputs["Wq_u"], f)[hs64].T.astype(bf)),
        "wrqT": np.ascontiguousarray(
            np.asarray(inputs["Wrq"], f)[hsl64].T.astype(bf)),
        "wvuT": np.ascontiguousarray(
            np.asarray(inputs["Wv_u"], f)[hs128].T.astype(bf)),
        "woT": np.ascontiguousarray(
            np.asarray(inputs["Wo"], f).T[hs128].astype(bf)),
        "bkvd": np.ascontiguousarray(
            np.asarray(inputs["bkv_d"], f).reshape(NL, 128).T),
        "bqd": np.ascontiguousarray(
            np.asarray(inputs["bq_d"], f).reshape(NL, 128).T),
        "bku": np.ascontiguousarray(
            np.asarray(inputs["bk_u"], f)[hs64].reshape(2, 128).T),
        "bqu": np.ascontiguousarray(
            np.asarray(inputs["bq_u"], f)[hs64].reshape(2, 128).T),
        "brk": np.ascontiguousarray(
            np.asarray(inputs["brk"], f)[hsl64].reshape(2, 128).T),
        "brq": np.ascontiguousarray(
            np.asarray(inputs["brq"], f)[hsl64].reshape(2, 128).T),
        "bvu": np.ascontiguousarray(
            np.asarray(inputs["bv_u"], f)[hs128].reshape(1, HPC * D)),
        "onesd": np.ones((128, 1), dtype=bf),
        "cosT": cosT.astype(bf),
        "sinT": sinT.astype(bf),
    }
    return im


def _get_runtime():
    if "rt" in _RT:
        return _RT["rt"]
    import jax
    import numpy as _np
    from jax.sharding import Mesh, PartitionSpec
    from jax.experimental.shard_map import shard_map

    import concourse.mybir as mybir
    from concourse import bass2jax

    nc = _build_program()
    bass2jax.install_neuronx_cc_hook()

    partition_name = (nc.partition_id_tensor.name
                      if nc.partition_id_tensor else None)
    in_names, out_names, out_avals, zero_shapes = [], [], [], []
    for alloc in nc.m.functions[0].allocations:
        if not isinstance(alloc, mybir.MemoryLocationSet):
            continue
        name = alloc.memorylocations[0].name
        if alloc.kind == "ExternalInput":
            if name != partition_name:
                in_names.append(name)
        elif alloc.kind == "ExternalOutput":
            out_names.append(name)
            np_dt = mybir.dt.np(alloc.dtype)
            out_avals.append(jax.core.ShapedArray(
                tuple(alloc.tensor_shape), np_dt))
            zero_shapes.append((tuple(alloc.tensor_shape), np_dt))

    n_params = len(in_names)
    n_outs = len(out_names)
    all_in_names = list(in_names) + list(out_names)
    if partition_name is not None:
        all_in_names.append(partition_name)

    def _body(*args):
        operands = list(args)
        if partition_name is not None:
            operands.append(bass2jax.partition_id_tensor())
        outs = bass2jax._bass_exec_p.bind(
            *operands,
            out_avals=tuple(out_avals),
            in_names=tuple(all_in_names),
            out_names=tuple(out_names),
            lowering_input_output_aliases=(),
            sim_require_finite=True,
            sim_require_nnan=True,
            nc=nc,
        )
        return tuple(outs)

    devices = jax.devices()[:NCORES]
    mesh = Mesh(_np.asarray(devices), ("core",))
    in_specs = (PartitionSpec("core"),) * (n_params + n_outs)
    out_specs = (PartitionSpec("core"),) * n_outs
    donate = tuple(range(n_params, n_params + n_outs))
    sharded = jax.jit(
        shard_map(_body, mesh=mesh, in_specs=in_specs, out_specs=out_specs,
                  check_rep=False),
        donate_argnums=donate, keep_unused=True)

    _RT["rt"] = dict(nc=nc, sharded=sharded, in_names=in_names,
                     out_names=out_names, zero_shapes=zero_shapes,
                     n_outs=n_outs)
    return _RT["rt"]


def _run_cores(in_maps):
    rt = _get_runtime()
    import numpy as _np
    concat_in = [
        _np.concatenate([in_maps[c][name] for c in range(NCORES)], axis=0)
        for name in rt["in_names"]
    ]
    concat_zeros = [
        _np.zeros((NCORES * shp[0],) + shp[1:], dt)
        for (shp, dt) in rt["zero_shapes"]
    ]
    out_arrs = rt["sharded"](*concat_in, *concat_zeros)
    res = []
    for c in range(NCORES):
        m = {}
        for i, name in enumerate(rt["out_names"]):
            shp, dt = rt["zero_shapes"][i]
            m[name] = _np.asarray(out_arrs[i]).reshape((NCORES,) + shp)[c]
        res.append(m)
    return res


def kernel(**inputs):
    in_maps = [_per_core_inputs(inputs, c) for c in range(NCORES)]
    res = _run_cores(in_maps)
    bo = np.asarray(inputs["bo"], dtype=np.float32)
    final = np.empty((B, S, E), dtype=np.float32)
    for b in range(B):
        acc = res[HPC * b]["out"].astype(np.float32).copy()
        for g in range(1, HPC):
            acc += res[HPC * b + g]["out"]
        final[b] = acc + bo[None, :]
    return final


# revision 31
# speedup vs baseline: 1.1268x; 1.0700x over previous
"""MultiHeadLatentAttention TRN2 kernel (bf16 data path).

Sharding: 8 cores = 2 (batch) x 4 (head groups of 4 heads).
Each core computes, for its batch b and heads hg*4..hg*4+3:
  - latent down-projections kv_d, q_d (replicated within the batch group)
  - per-head up-projections K^T, Q^T (with RoPE), V
  - full attention for its 4 heads
  - partial output projection (its 512 columns of Wo's input dim)
Partial outputs are summed on the host (+ bo).

All matmul operands are bf16 (fp32 PSUM accumulation), which runs the
PE at full rate with fast weight loads and halves DMA/SBUF traffic.
Big tensors live in "feature-on-partitions" (transposed) layout so
every matmul has free dim 512.
RoPE's rotate_half is a partition-pair swap: the rope feature rows are
stored in host-permuted order (pairs (i, i+32) adjacent) so DVE
stream_shuffle(mask=i^1) implements the rotation; the sign lives in the
host-built sin table.
Softmax skips max-subtraction (scores are bounded, exp is safe);
row sums accumulate in PSUM via per-chunk ones-matmuls; reciprocals
are batched into one ACT Reciprocal at the end of the attention phase
(single table switch), then broadcast and applied to the unnormalized
attention outputs.
"""

import sys

sys.path.insert(0, "/opt/trn_rl_repo")

from contextlib import ExitStack

import numpy as np

H = 16
E = 2048
LAT = E // 4          # 512
D = E // H            # 128
R = D // 2            # 64
B, S = 2, 2048
HPC = H // 4          # 4 heads per core
NCORES = 8
NE = E // 128         # 16 contraction chunks over E
NL = LAT // 128       # 4 contraction chunks over LAT
SW = 512              # s-chunk width for projections
NSC = S // SW         # 4 s-chunks
NKC = S // 128        # 16 key chunks
QW = 1024             # q-block width in attention
NQB = S // QW         # 2 q-blocks per head
NBLK = HPC * NQB      # 8 attention blocks per core
SCALE = 1.0 / float(np.sqrt(D))
LAG = 3               # PV trails QK/exp by LAG k-chunks
DEBUG_DUMPS = False   # extra ExternalOutputs with intermediates

_RT = {}  # cached runtimes


def _mk(nc):
    """Declare DRAM I/O; returns dict of handles."""
    import concourse.mybir as mybir
    F32 = mybir.dt.float32
    BF16 = mybir.dt.bfloat16
    d = {}
    F8 = mybir.dt.float8e4
    d["xT"] = nc.dram_tensor("xT", [E, S], BF16, kind="ExternalInput")
    # fp8 copy of x, pre-scaled by 1/8 (weights carry the 8x) so both
    # operands sit in e4m3's normal range; used by the logits-only
    # q_d / rope-k down-projections
    d["xT8"] = nc.dram_tensor("xT8", [E, S], F8, kind="ExternalInput")
    # down-proj weights packed [m*128+p, e*128+c] (p = in-feature within
    # e-chunk on partitions, c = out-feature within m-chunk)
    d["wkvd_p"] = nc.dram_tensor("wkvd_p", [NL * 128, NE * 128], BF16,
                                 kind="ExternalInput")
    d["wqd_p8"] = nc.dram_tensor("wqd_p8", [NL * 128, NE * 128], F8,
                                 kind="ExternalInput")
    d["wrk_p8"] = nc.dram_tensor("wrk_p8", [2 * 128, NE * 128], F8,
                                 kind="ExternalInput")
    d["wkuT"] = nc.dram_tensor("wkuT", [LAT, HPC * R], BF16,
                               kind="ExternalInput")
    d["wquT"] = nc.dram_tensor("wquT", [LAT, HPC * R], BF16,
                               kind="ExternalInput")
    d["wrqT"] = nc.dram_tensor("wrqT", [LAT, HPC * R], BF16,
                               kind="ExternalInput")
    d["wvuT"] = nc.dram_tensor("wvuT", [LAT, HPC * D], BF16,
                               kind="ExternalInput")
    d["woT"] = nc.dram_tensor("woT", [HPC * D, E], BF16,
                              kind="ExternalInput")
    d["bkvd"] = nc.dram_tensor("bkvd", [128, NL], F32, kind="ExternalInput")
    d["bqd"] = nc.dram_tensor("bqd", [128, NL], F32, kind="ExternalInput")
    d["bku"] = nc.dram_tensor("bku", [128, 2], F32, kind="ExternalInput")
    d["bqu"] = nc.dram_tensor("bqu", [128, 2], F32, kind="ExternalInput")
    d["brk"] = nc.dram_tensor("brk", [128, 2], F32, kind="ExternalInput")
    d["brq"] = nc.dram_tensor("brq", [128, 2], F32, kind="ExternalInput")
    d["bvu"] = nc.dram_tensor("bvu", [1, HPC * D], F32, kind="ExternalInput")
    d["onesd"] = nc.dram_tensor("onesd", [128, 1], BF16,
                                kind="ExternalInput")
    d["cosT"] = nc.dram_tensor("cosT", [128, S], BF16, kind="ExternalInput")
    d["sinT"] = nc.dram_tensor("sinT", [128, S], BF16, kind="ExternalInput")
    d["out"] = nc.dram_tensor("out", [S, E], BF16,
                          kind="ExternalOutput")
    if DEBUG_DUMPS:
        d["dbg_sums"] = nc.dram_tensor("dbg_sums", [1, NBLK * QW], F32,
                                       kind="ExternalOutput")
        d["dbg_recip"] = nc.dram_tensor("dbg_recip", [1, NBLK * QW], BF16,
                                        kind="ExternalOutput")
        d["dbg_K"] = nc.dram_tensor("dbg_K", [128, S], BF16,
                                    kind="ExternalOutput")
        d["dbg_Q"] = nc.dram_tensor("dbg_Q", [128, S], BF16,
                                    kind="ExternalOutput")
        d["dbg_V"] = nc.dram_tensor("dbg_V", [128, HPC * D], BF16,
                                    kind="ExternalOutput")
        d["dbg_att"] = nc.dram_tensor("dbg_att", [128, S], BF16,
                                      kind="ExternalOutput")
    return d


def _consts(nc, tc, top, d):
    """Persistent tiles: K/Q/V/att storage, biases, ones, up/out weights."""
    import concourse.mybir as mybir
    F32 = mybir.dt.float32
    BF16 = mybir.dt.bfloat16

    kq_pool = top.enter_context(tc.tile_pool(name="kq", bufs=1))
    v_pool = top.enter_context(tc.tile_pool(name="vp", bufs=1))
    att_pool = top.enter_context(tc.tile_pool(name="att", bufs=1))
    cpool = top.enter_context(tc.tile_pool(name="cp", bufs=1))

    t = {}
    t["K"] = [kq_pool.tile([128, S], BF16, name=f"Kt{h}") for h in range(HPC)]
    t["Q"] = [kq_pool.tile([128, S], BF16, name=f"Qt{h}") for h in range(HPC)]
    t["V"] = [v_pool.tile([128, HPC * D], BF16, name=f"Vt{i}")
              for i in range(NKC)]
    t["att"] = [att_pool.tile([128, S], BF16, name=f"att{h}")
                for h in range(HPC)]

    def ld(name, dram, shape, dt=F32):
        tl = cpool.tile(shape, dt, name=name)
        nc.sync.dma_start(tl[:], dram[:])
        return tl

    t["ones"] = ld("ones_t", d["onesd"], [128, 1], BF16)
    t["bkvd"] = ld("bkvd_t", d["bkvd"], [128, NL])
    t["bqd"] = ld("bqd_t", d["bqd"], [128, NL])
    t["bku"] = ld("bku_t", d["bku"], [128, 2])
    t["bqu"] = ld("bqu_t", d["bqu"], [128, 2])
    t["brk"] = ld("brk_t", d["brk"], [128, 2])
    t["brq"] = ld("brq_t", d["brq"], [128, 2])

    # up-weights + rope tables: tiles allocated here, DMAs emitted inside
    # _emit_A after the first s-chunk's x/weight loads are queued, so the
    # first down-projection matmuls start as early as possible.
    upw = {}
    for nm, w in (("ku", HPC * R), ("qu", HPC * R),
                  ("rq", HPC * R), ("vu", HPC * D)):
        upw[nm] = [cpool.tile([128, w], BF16, name=f"w{nm}{l}")
                   for l in range(NL)]
    t["upw"] = upw
    t["cos"] = cpool.tile([128, S], BF16, name="cos_t")
    t["sin"] = cpool.tile([128, S], BF16, name="sin_t")
    bvu_row = ld("bvu_row", d["bvu"], [1, HPC * D])
    bvu_bc = cpool.tile([128, HPC * D], F32, name="bvu_bc")
    nc.gpsimd.partition_broadcast(bvu_bc[:], bvu_row[:])
    t["bvu_bc"] = bvu_bc

    # Wo streams in late (only needed in phase C)
    wo_t = [cpool.tile([128, E], BF16, name=f"wo{hc}") for hc in range(HPC)]
    t["wo"] = wo_t
    t["wo_dram"] = d["woT"]
    return t


def _phaseA_pools(tc, pa):
    p = {}
    p["xa"] = pa.enter_context(tc.tile_pool(name="xa", bufs=2))
    p["wa"] = pa.enter_context(tc.tile_pool(name="wa", bufs=3))
    p["kvq"] = pa.enter_context(tc.tile_pool(name="kvq", bufs=1))
    p["rp"] = pa.enter_context(tc.tile_pool(name="rp", bufs=2))
    p["psA"] = pa.enter_context(tc.tile_pool(name="psA", bufs=4,
                                             space="PSUM"))
    return p


def _emit_A(nc, tc, d, t, p):
    import concourse.mybir as mybir
    from concourse.alu_op_type import AluOpType
    F32 = mybir.dt.float32
    BF16 = mybir.dt.bfloat16
    F8 = mybir.dt.float8e4
    DR = mybir.MatmulPerfMode.DoubleRow
    K_t, Q_t, V_t, upw = t["K"], t["Q"], t["V"], t["upw"]
    swap_mask = [i ^ 1 for i in range(32)]

    # DMA issue spread over otherwise-idle engine queues for parallelism
    qeng = [nc.sync, nc.gpsimd, nc.scalar]

    for sc in range(NSC):
        ssl = slice(sc * SW, (sc + 1) * SW)
        EPD = 2   # e-chunks per tile/dma
        xts = []
        xts8 = []
        for i in range(NE // EPD):
            esl = slice(i * EPD * 128, (i + 1) * EPD * 128)
            xi = p["xa"].tile([128, EPD * SW], BF16, name=f"xt{i}")
            qeng[i % 3].dma_start(
                xi[:].rearrange("p (e s) -> p e s", e=EPD),
                d["xT"][esl, ssl].rearrange("(e p) s -> p e s", p=128))
            xts.append(xi)

        def xchunk(e):
            return xts[e // EPD][:, (e % EPD) * SW:(e % EPD + 1) * SW]

        def down_mm(wdram, m):
            wt = p["wa"].tile([128, NE * 128], BF16, name="wt")
            half = NE * 128 // 2
            nc.sync.dma_start(wt[:, 0:half],
                              wdram[m * 128:(m + 1) * 128, 0:half])
            nc.gpsimd.dma_start(wt[:, half:],
                                wdram[m * 128:(m + 1) * 128, half:])
            ps = p["psA"].tile([128, SW], F32, name="psA_t")
            for e in range(NE):
                nc.tensor.matmul(ps[:], wt[:, e * 128:(e + 1) * 128],
                                 xchunk(e),
                                 start=(e == 0), stop=(e == NE - 1))
            return ps

        def down_mm8(wdram, m):
            # fp8 DoubleRow: each matmul contracts an e-chunk PAIR via
            # 3D APs [p, 2, f]
            wt8 = p["wa"].tile([128, NE * 128], F8, name="wt8")
            half = NE * 128 // 2
            nc.sync.dma_start(wt8[:, 0:half],
                              wdram[m * 128:(m + 1) * 128, 0:half])
            nc.gpsimd.dma_start(wt8[:, half:],
                                wdram[m * 128:(m + 1) * 128, half:])
            w3 = wt8[:].rearrange("p (e c) -> p e c", e=NE)
            ps = p["psA"].tile([128, SW], F32, name="psA_t")
            for i in range(NE // 2):
                nc.tensor.matmul(
                    ps[:], w3[:, 2 * i:2 * i + 2, :],
                    xts8[i][:].rearrange("p (e s) -> p e s", e=2),
                    start=(i == 0), stop=(i == NE // 2 - 1),
                    perf_mode=DR)
            return ps

        def rope(ps, bias_t, m, dst):
            # ps: [128 rows = 2 heads x 64 rope rows, SW]
            xb = p["rp"].tile([128, SW], BF16, name="xb")
            nc.vector.tensor_scalar_add(xb[:], ps[:], bias_t[:, m:m + 1])
            sh = p["rp"].tile([128, SW], BF16, name="sh")
            nc.vector.stream_shuffle(sh[:], xb[:], swap_mask)
            t1 = p["rp"].tile([128, SW], BF16, name="t1")
            nc.vector.tensor_tensor(t1[:], xb[:], t["cos"][:, ssl],
                                    op=AluOpType.mult)
            t2 = p["rp"].tile([128, SW], BF16, name="t2")
            nc.vector.tensor_tensor(t2[:], sh[:], t["sin"][:, ssl],
                                    op=AluOpType.mult)
            nc.vector.tensor_tensor(dst[2 * m][R:D, ssl], t1[0:R, :],
                                    t2[0:R, :], op=AluOpType.add)
            nc.vector.tensor_tensor(dst[2 * m + 1][R:D, ssl], t1[R:D, :],
                                    t2[R:D, :], op=AluOpType.add)

        def up_mm(src, w, m):
            ps = p["psA"].tile([128, SW], F32, name="psA_t")
            for l in range(NL):
                nc.tensor.matmul(ps[:], w[l][:, m * 128:(m + 1) * 128],
                                 src[l][:], start=(l == 0),
                                 stop=(l == NL - 1))
            return ps

        # latent kv_d down-projection (replicated in batch group)
        kv_s = []
        for m in range(NL):
            ps = down_mm(d["wkvd_p"], m)
            tl = p["kvq"].tile([128, SW], BF16, name=f"lat{m}")
            nc.scalar.add(tl[:], ps[:], t["bkvd"][:, m:m + 1])
            kv_s.append(tl)
        # x8 chunks for the fp8 q-side: loaded after the kv stream so
        # they don't congest the startup queues (first use is ~40us in)
        for i in range(NE // EPD):
            esl = slice(i * EPD * 128, (i + 1) * EPD * 128)
            x8 = p["xa"].tile([128, EPD * SW], F8, name=f"xt8_{i}")
            qeng[(i + 1) % 3].dma_start(
                x8[:].rearrange("p (e s) -> p e s", e=EPD),
                d["xT8"][esl, ssl].rearrange("(e p) s -> p e s", p=128))
            xts8.append(x8)
        if sc == 0:
            # stream in the up-weights + rope tables behind the first
            # down-projection's loads
            for nm, key in (("ku", "wkuT"), ("qu", "wquT"),
                            ("rq", "wrqT"), ("vu", "wvuT")):
                for l in range(NL):
                    qeng[l % 3].dma_start(
                        t["upw"][nm][l][:],
                        d[key][l * 128:(l + 1) * 128, :])
            nc.scalar.dma_start(t["cos"][:], d["cosT"][:])
            nc.scalar.dma_start(t["sin"][:], d["sinT"][:])
        for m in range(2):  # k1 -> K rows 0..63
            ps = up_mm(kv_s, upw["ku"], m)
            nc.vector.tensor_scalar_add(K_t[2 * m][0:R, ssl], ps[0:R, :],
                                        t["bku"][0:R, m:m + 1])
            nc.vector.tensor_scalar_add(K_t[2 * m + 1][0:R, ssl], ps[R:D, :],
                                        t["bku"][R:D, m:m + 1])
        for j in range(SW // 128):  # V, (s, feat) layout
            ps = p["psA"].tile([128, HPC * D], F32, name="psV_t")
            for l in range(NL):
                nc.tensor.matmul(ps[:], kv_s[l][:, j * 128:(j + 1) * 128],
                                 upw["vu"][l][:], start=(l == 0),
                                 stop=(l == NL - 1))
            nc.vector.tensor_tensor(V_t[sc * (SW // 128) + j][:], ps[:],
                                    t["bvu_bc"][:], op=AluOpType.add)

        # latent q_d down-projection (fp8 DoubleRow, logits-only path)
        q_s = []
        for m in range(NL):
            ps = down_mm8(d["wqd_p8"], m)
            tl = p["kvq"].tile([128, SW], BF16, name=f"latq{m}")
            nc.scalar.add(tl[:], ps[:], t["bqd"][:, m:m + 1])
            q_s.append(tl)
        for m in range(2):  # q1 -> Q rows 0..63
            ps = up_mm(q_s, upw["qu"], m)
            nc.vector.tensor_scalar_add(Q_t[2 * m][0:R, ssl], ps[0:R, :],
                                        t["bqu"][0:R, m:m + 1])
            nc.vector.tensor_scalar_add(Q_t[2 * m + 1][0:R, ssl], ps[R:D, :],
                                        t["bqu"][R:D, m:m + 1])
        for m in range(2):  # rope-q from q_d
            ps = up_mm(q_s, upw["rq"], m)
            rope(ps, t["brq"], m, Q_t)
        # rope-k from x (fp8 DoubleRow, logits-only path)
        for m in range(2):
            ps = down_mm8(d["wrk_p8"], m)
            rope(ps, t["brk"], m, K_t)


def _phaseB_pools(tc, pb):
    p = {}
    p["pe"] = pb.enter_context(tc.tile_pool(name="pe", bufs=6))
    p["sm"] = pb.enter_context(tc.tile_pool(name="sm", bufs=2))
    p["dr"] = pb.enter_context(tc.tile_pool(name="dr", bufs=2,
                                            space="DRAM"))
    p["cb"] = pb.enter_context(tc.tile_pool(name="cb", bufs=2))
    p["psS"] = pb.enter_context(tc.tile_pool(name="psS", bufs=2,
                                             space="PSUM"))
    p["psO"] = pb.enter_context(tc.tile_pool(name="psO", bufs=1,
                                             space="PSUM"))
    p["psR"] = pb.enter_context(tc.tile_pool(name="psR", bufs=1,
                                             space="PSUM"))
    return p


def _emit_B(nc, tc, d, t, p):
    import concourse.mybir as mybir
    from concourse.alu_op_type import AluOpType
    F32 = mybir.dt.float32
    BF16 = mybir.dt.bfloat16
    AF = mybir.ActivationFunctionType
    K_t, Q_t, V_t, att_t = t["K"], t["Q"], t["V"], t["att"]

    GRP = 4   # pe chunks pre-reduced on DVE before each sum matmul
    NCOL = QW // 128

    for h in range(HPC):
        for qp in range(NQB):
            blk = h * NQB + qp
            qa = slice(qp * QW, qp * QW + 512)
            qb = slice(qp * QW + 512, (qp + 1) * QW)
            qsl = slice(qp * QW, (qp + 1) * QW)
            oA = p["psO"].tile([128, 512], F32, name="oA")
            oB = p["psO"].tile([128, 512], F32, name="oB")
            s0 = p["psR"].tile([1, 512], F32, name="s0")
            s1 = p["psR"].tile([1, 512], F32, name="s1")
            pes = {}

            def pv(kk):
                pe = pes.pop(kk)
                nc.tensor.matmul(oA[:], V_t[kk][:, h * D:(h + 1) * D],
                                 pe[:, 0:512], start=(kk == 0),
                                 stop=(kk == NKC - 1))
                nc.tensor.matmul(oB[:], V_t[kk][:, h * D:(h + 1) * D],
                                 pe[:, 512:1024], start=(kk == 0),
                                 stop=(kk == NKC - 1))

            live = {}
            for kk in range(NKC):
                ksl = slice(kk * 128, (kk + 1) * 128)
                pp = p["psS"].tile([128, 1024], F32, name="pp")
                nc.tensor.matmul(pp[:, 0:512], K_t[h][:, ksl], Q_t[h][:, qa],
                                 start=True, stop=True)
                nc.tensor.matmul(pp[:, 512:1024], K_t[h][:, ksl],
                                 Q_t[h][:, qb], start=True, stop=True)
                pe = p["pe"].tile([128, 1024], BF16, name="pet")
                nc.scalar.activation(pe[:], pp[:], AF.Exp, scale=SCALE)
                pes[kk] = pe
                live[kk] = pe
                # 4-way DVE pre-reduction, then one accumulating ones-matmul
                if kk % GRP == GRP - 1:
                    g = kk // GRP
                    a01 = p["pe"].tile([128, 1024], BF16, name="tadd0",
                                       bufs=2)
                    a23 = p["pe"].tile([128, 1024], BF16, name="tadd1",
                                       bufs=2)
                    a03 = p["pe"].tile([128, 1024], BF16, name="tadd2",
                                       bufs=2)
                    with nc.allow_low_precision(
                            reason="4-term bf16 pre-reduction of exp "
                                   "chunks; fp32 PSUM accumulates groups"):
                        nc.vector.tensor_tensor(a01[:], live[kk - 3][:],
                                                live[kk - 2][:],
                                                op=AluOpType.add)
                        nc.vector.tensor_tensor(a23[:], live[kk - 1][:],
                                                live[kk][:],
                                                op=AluOpType.add)
                        nc.vector.tensor_tensor(a03[:], a01[:], a23[:],
                                                op=AluOpType.add)
                    live.clear()
                    nc.tensor.matmul(s0[:], t["ones"][:], a03[:, 0:512],
                                     start=(g == 0),
                                     stop=(g == NKC // GRP - 1))
                    nc.tensor.matmul(s1[:], t["ones"][:], a03[:, 512:1024],
                                     start=(g == 0),
                                     stop=(g == NKC // GRP - 1))
                if kk >= LAG:
                    pv(kk - LAG)
            for kk in range(NKC - LAG, NKC):
                pv(kk)
            # stash unnormalized attention output
            nc.vector.tensor_copy(att_t[h][:, qa], oA[:])
            nc.vector.tensor_copy(att_t[h][:, qb], oB[:])
            # per-block reciprocal pipeline: sums row -> DRAM ->
            # [128, 8] spread -> DVE reciprocal -> DRAM -> row ->
            # broadcast -> normalize.  Overlaps the next block's matmuls.
            srow = p["sm"].tile([1, QW], F32, name="srow")
            nc.vector.tensor_copy(srow[:, 0:512], s0[:])
            nc.vector.tensor_copy(srow[:, 512:QW], s1[:])
            dram_row = p["dr"].tile([1, QW], F32, name="dram_row")
            nc.sync.dma_start(dram_row[:], srow[:])
            rs2 = p["sm"].tile([128, NCOL], F32, name="rs2")
            nc.sync.dma_start(
                rs2[:], dram_row[:].rearrange("o (p c) -> (o p) c", p=128))
            rr2 = p["sm"].tile([128, NCOL], F32, name="rr2")
            nc.vector.reciprocal(rr2[:], rs2[:])
            rr2b = p["sm"].tile([128, NCOL], BF16, name="rr2b")
            nc.vector.tensor_copy(rr2b[:], rr2[:])
            dram_rb = p["dr"].tile([1, QW], BF16, name="dram_rb")
            nc.sync.dma_start(
                dram_rb[:].rearrange("o (p c) -> (o p) c", p=128), rr2b[:])
            recip = p["sm"].tile([1, QW], BF16, name="recip")
            nc.sync.dma_start(recip[:], dram_rb[:])
            cb = p["cb"].tile([128, QW], BF16, name="cbt")
            nc.gpsimd.partition_broadcast(cb[:], recip[:])
            nc.vector.tensor_tensor(att_t[h][:, qsl], att_t[h][:, qsl],
                                    cb[:], op=AluOpType.mult)
            if DEBUG_DUMPS and blk < 2:
                nc.sync.dma_start(
                    d["dbg_sums"][:, blk * QW:(blk + 1) * QW], srow[:])
                nc.sync.dma_start(
                    d["dbg_recip"][:, blk * QW:(blk + 1) * QW], recip[:])
    if DEBUG_DUMPS:
        nc.sync.dma_start(d["dbg_K"][:], t["K"][0][:])
        nc.sync.dma_start(d["dbg_Q"][:], t["Q"][0][:])
        nc.sync.dma_start(d["dbg_V"][:], t["V"][0][:])
        nc.sync.dma_start(d["dbg_att"][:], t["att"][0][:])


def _phaseC_pools(tc, pc):
    p = {}
    p["oc"] = pc.enter_context(tc.tile_pool(name="oc", bufs=4))
    p["psC"] = pc.enter_context(tc.tile_pool(name="psC", bufs=8,
                                             space="PSUM"))
    return p


def _emit_C(nc, tc, d, t, p):
    import concourse.mybir as mybir
    F32 = mybir.dt.float32
    BF16 = mybir.dt.bfloat16
    att_t, wo_t = t["att"], t["wo"]

    qeng = [nc.sync, nc.gpsimd, nc.scalar]
    for hc in range(HPC):
        qeng[hc % 3].dma_start(wo_t[hc][:],
                               t["wo_dram"][hc * 128:(hc + 1) * 128, :])

    for sj in range(S // 128):
        pss = [p["psC"].tile([128, 512], F32, name="psC_t")
               for _ in range(E // 512)]
        for hc in range(HPC):
            for ocn in range(E // 512):
                nc.tensor.matmul(pss[ocn][:],
                                 att_t[hc][:, sj * 128:(sj + 1) * 128],
                                 wo_t[hc][:, ocn * 512:(ocn + 1) * 512],
                                 start=(hc == 0), stop=(hc == HPC - 1))
        for ocn in range(E // 512):
            ob = p["oc"].tile([128, 512], BF16, name="ob")
            if ocn % 2 == 0:
                nc.vector.tensor_copy(ob[:], pss[ocn][:])
            else:
                nc.scalar.copy(ob[:], pss[ocn][:])
            qeng[(sj * 4 + ocn) % 3].dma_start(
                d["out"][sj * 128:(sj + 1) * 128,
                         ocn * 512:(ocn + 1) * 512], ob[:])


def _build_program():
    import concourse.bacc as bacc
    import concourse.tile as tile

    nc = bacc.Bacc("TRN2", target_bir_lowering=False, debug=False,
                   num_devices=NCORES)
    d = _mk(nc)

    with tile.TileContext(nc) as tc, ExitStack() as top:
        t = _consts(nc, tc, top, d)
        with ExitStack() as pa:
            pA = _phaseA_pools(tc, pa)
            _emit_A(nc, tc, d, t, pA)
        with ExitStack() as pb:
            pB = _phaseB_pools(tc, pb)
            _emit_B(nc, tc, d, t, pB)
        with ExitStack() as pc:
            pC = _phaseC_pools(tc, pc)
            _emit_C(nc, tc, d, t, pC)

    nc.compile()
    return nc


def _rope_tables():
    inv_freq = 1.0 / (10000.0 ** (np.arange(0, R, 2, dtype=np.float64) / R))
    t = np.arange(S, dtype=np.float64)
    freqs = np.outer(t, inv_freq)                       # (S, R/2)
    emb = np.concatenate([freqs, freqs], axis=-1)       # (S, R)
    cos = np.cos(emb).astype(np.float32)                # (S, R)
    sin = np.sin(emb).astype(np.float32)
    perm = np.array([(j // 2) if j % 2 == 0 else (j // 2) + R // 2
                     for j in range(R)])
    sign = np.array([-1.0 if j % 2 == 0 else 1.0
                     for j in range(R)], dtype=np.float32)
    cos_p = cos[:, perm].T.copy()                       # (R, S)
    sin_p = (sin[:, perm] * sign[None, :]).T.copy()     # (R, S)
    cosT = np.concatenate([cos_p, cos_p], axis=0)       # (128, S)
    sinT = np.concatenate([sin_p, sin_p], axis=0)
    return cosT, sinT, perm


def _pack_down(Wm, nm, dt=None):
    """Pack [nm*128, E] weight into [m*128+p, e*128+c] layout."""
    import ml_dtypes
    if dt is None:
        dt = ml_dtypes.bfloat16
    a = Wm.reshape(nm, 128, NE, 128)        # [m, c, e, p]
    a = a.transpose(0, 3, 2, 1)             # [m, p, e, c]
    return np.ascontiguousarray(
        a.reshape(nm * 128, NE * 128).astype(dt))


def _per_core_inputs(inputs, core):
    import ml_dtypes
    bf = ml_dtypes.bfloat16
    b, hg = divmod(core, HPC)
    cosT, sinT, perm = _rope_tables()
    hsl64 = np.concatenate([hg * HPC * R + h * R + perm
                            for h in range(HPC)])       # permuted rope rows
    hs64 = slice(hg * HPC * R, (hg + 1) * HPC * R)      # natural 64-rows
    hs128 = slice(hg * HPC * D, (hg + 1) * HPC * D)     # natural 128-rows

    x = np.asarray(inputs["x"], dtype=np.float32)
    f = np.float32
    e4 = ml_dtypes.float8_e4m3
    im = {
        "xT": np.ascontiguousarray(x[b].T.astype(bf)),
        "xT8": np.ascontiguousarray((x[b].T / 8.0).astype(e4)),
        "wkvd_p": _pack_down(np.asarray(inputs["Wkv_d"], f), NL),
        "wqd_p8": _pack_down(8.0 * np.asarray(inputs["Wq_d"], f), NL, e4),
        "wrk_p8": _pack_down(8.0 * np.asarray(inputs["Wrk"], f)[hsl64], 2,
                             e4),
        "wkuT": np.ascontiguousarray(
            np.asarray(inputs["Wk_u"], f)[hs64].T.astype(bf)),
        "wquT": np.ascontiguousarray(
            np.asarray(inputs["Wq_u"], f)[hs64].T.astype(bf)),
        "wrqT": np.ascontiguousarray(
            np.asarray(inputs["Wrq"], f)[hsl64].T.astype(bf)),
        "wvuT": np.ascontiguousarray(
            np.asarray(inputs["Wv_u"], f)[hs128].T.astype(bf)),
        "woT": np.ascontiguousarray(
            np.asarray(inputs["Wo"], f).T[hs128].astype(bf)),
        "bkvd": np.ascontiguousarray(
            np.asarray(inputs["bkv_d"], f).reshape(NL, 128).T),
        "bqd": np.ascontiguousarray(
            np.asarray(inputs["bq_d"], f).reshape(NL, 128).T),
        "bku": np.ascontiguousarray(
            np.asarray(inputs["bk_u"], f)[hs64].reshape(2, 128).T),
        "bqu": np.ascontiguousarray(
            np.asarray(inputs["bq_u"], f)[hs64].reshape(2, 128).T),
        "brk": np.ascontiguousarray(
            np.asarray(inputs["brk"], f)[hsl64].reshape(2, 128).T),
        "brq": np.ascontiguousarray(
            np.asarray(inputs["brq"], f)[hsl64].reshape(2, 128).T),
        "bvu": np.ascontiguousarray(
            np.asarray(inputs["bv_u"], f)[hs128].reshape(1, HPC * D)),
        "onesd": np.ones((128, 1), dtype=bf),
        "cosT": cosT.astype(bf),
        "sinT": sinT.astype(bf),
    }
    return im


def _get_runtime():
    if "rt" in _RT:
        return _RT["rt"]
    import jax
    import numpy as _np
    from jax.sharding import Mesh, PartitionSpec
    from jax.experimental.shard_map import shard_map

    import concourse.mybir as mybir
    from concourse import bass2jax

    nc = _build_program()
    bass2jax.install_neuronx_cc_hook()

    partition_name = (nc.partition_id_tensor.name
                      if nc.partition_id_tensor else None)
    in_names, out_names, out_avals, zero_shapes = [], [], [], []
    for alloc in nc.m.functions[0].allocations:
        if not isinstance(alloc, mybir.MemoryLocationSet):
            continue
        name = alloc.memorylocations[0].name
        if alloc.kind == "ExternalInput":
            if name != partition_name:
                in_names.append(name)
        elif alloc.kind == "ExternalOutput":
            out_names.append(name)
            np_dt = mybir.dt.np(alloc.dtype)
            out_avals.append(jax.core.ShapedArray(
                tuple(alloc.tensor_shape), np_dt))
            zero_shapes.append((tuple(alloc.tensor_shape), np_dt))

    n_params = len(in_names)
    n_outs = len(out_names)
    all_in_names = list(in_names) + list(out_names)
    if partition_name is not None:
        all_in_names.append(partition_name)

    def _body(*args):
        operands = list(args)
        if partition_name is not None:
            operands.append(bass2jax.partition_id_tensor())
        outs = bass2jax._bass_exec_p.bind(
            *operands,
            out_avals=tuple(out_avals),
            in_names=tuple(all_in_names),
            out_names=tuple(out_names),
            lowering_input_output_aliases=(),
            sim_require_finite=True,
            sim_require_nnan=True,
            nc=nc,
        )
        return tuple(outs)

    devices = jax.devices()[:NCORES]
    mesh = Mesh(_np.asarray(devices), ("core",))
    in_specs = (PartitionSpec("core"),) * (n_params + n_outs)
    out_specs = (PartitionSpec("core"),) * n_outs
    donate = tuple(range(n_params, n_params + n_outs))
    sharded = jax.jit(
        shard_map(_body, mesh=mesh, in_specs=in_specs, out_specs=out_specs,
                  check_rep=False),
        donate_argnums=donate, keep_unused=True)

    _RT["rt"] = dict(nc=nc, sharded=sharded, in_names=in_names,
                     out_names=out_names, zero_shapes=zero_shapes,
                     n_outs=n_outs)
    return _RT["rt"]


def _run_cores(in_maps):
    rt = _get_runtime()
    import numpy as _np
    concat_in = [
        _np.concatenate([in_maps[c][name] for c in range(NCORES)], axis=0)
        for name in rt["in_names"]
    ]
    concat_zeros = [
        _np.zeros((NCORES * shp[0],) + shp[1:], dt)
        for (shp, dt) in rt["zero_shapes"]
    ]
    out_arrs = rt["sharded"](*concat_in, *concat_zeros)
    res = []
    for c in range(NCORES):
        m = {}
        for i, name in enumerate(rt["out_names"]):
            shp, dt = rt["zero_shapes"][i]
            m[name] = _np.asarray(out_arrs[i]).reshape((NCORES,) + shp)[c]
        res.append(m)
    return res


def kernel(**inputs):
    in_maps = [_per_core_inputs(inputs, c) for c in range(NCORES)]
    res = _run_cores(in_maps)
    bo = np.asarray(inputs["bo"], dtype=np.float32)
    final = np.empty((B, S, E), dtype=np.float32)
    for b in range(B):
        acc = res[HPC * b]["out"].astype(np.float32)
        for g in range(1, HPC):
            acc = acc + res[HPC * b + g]["out"].astype(np.float32)
        final[b] = acc + bo[None, :]
    return final


# revision 32
# speedup vs baseline: 1.1412x; 1.0128x over previous
"""MultiHeadLatentAttention TRN2 kernel (bf16 data path).

Sharding: 8 cores = 2 (batch) x 4 (head groups of 4 heads).
Each core computes, for its batch b and heads hg*4..hg*4+3:
  - latent down-projections kv_d, q_d (replicated within the batch group)
  - per-head up-projections K^T, Q^T (with RoPE), V
  - full attention for its 4 heads
  - partial output projection (its 512 columns of Wo's input dim)
Partial outputs are summed on the host (+ bo).

All matmul operands are bf16 (fp32 PSUM accumulation), which runs the
PE at full rate with fast weight loads and halves DMA/SBUF traffic.
Big tensors live in "feature-on-partitions" (transposed) layout so
every matmul has free dim 512.
RoPE's rotate_half is a partition-pair swap: the rope feature rows are
stored in host-permuted order (pairs (i, i+32) adjacent) so DVE
stream_shuffle(mask=i^1) implements the rotation; the sign lives in the
host-built sin table.
Softmax skips max-subtraction (scores are bounded, exp is safe);
row sums accumulate in PSUM via per-chunk ones-matmuls; reciprocals
are batched into one ACT Reciprocal at the end of the attention phase
(single table switch), then broadcast and applied to the unnormalized
attention outputs.
"""

import sys

sys.path.insert(0, "/opt/trn_rl_repo")

from contextlib import ExitStack

import numpy as np

H = 16
E = 2048
LAT = E // 4          # 512
D = E // H            # 128
R = D // 2            # 64
B, S = 2, 2048
HPC = H // 4          # 4 heads per core
NCORES = 8
NE = E // 128         # 16 contraction chunks over E
NL = LAT // 128       # 4 contraction chunks over LAT
SW = 512              # s-chunk width for projections
NSC = S // SW         # 4 s-chunks
NKC = S // 128        # 16 key chunks
QW = 1024             # q-block width in attention
NQB = S // QW         # 2 q-blocks per head
NBLK = HPC * NQB      # 8 attention blocks per core
SCALE = 1.0 / float(np.sqrt(D))
LAG = 3               # PV trails QK/exp by LAG k-chunks
DEBUG_DUMPS = False   # extra ExternalOutputs with intermediates

_RT = {}  # cached runtimes


def _mk(nc):
    """Declare DRAM I/O; returns dict of handles."""
    import concourse.mybir as mybir
    F32 = mybir.dt.float32
    BF16 = mybir.dt.bfloat16
    d = {}
    F8 = mybir.dt.float8e4
    d["xT"] = nc.dram_tensor("xT", [E, S], BF16, kind="ExternalInput")
    # fp8 copy of x, pre-scaled by 1/8 (weights carry the 8x) so both
    # operands sit in e4m3's normal range; used by the logits-only
    # q_d / rope-k down-projections
    d["xT8"] = nc.dram_tensor("xT8", [E, S], F8, kind="ExternalInput")
    # down-proj weights packed [m*128+p, e*128+c] (p = in-feature within
    # e-chunk on partitions, c = out-feature within m-chunk)
    d["wkvd_p"] = nc.dram_tensor("wkvd_p", [NL * 128, NE * 128], BF16,
                                 kind="ExternalInput")
    d["wqd_p8"] = nc.dram_tensor("wqd_p8", [NL * 128, NE * 128], F8,
                                 kind="ExternalInput")
    d["wrk_p8"] = nc.dram_tensor("wrk_p8", [2 * 128, NE * 128], F8,
                                 kind="ExternalInput")
    d["wkuT"] = nc.dram_tensor("wkuT", [LAT, HPC * R], BF16,
                               kind="ExternalInput")
    d["wquT"] = nc.dram_tensor("wquT", [LAT, HPC * R], BF16,
                               kind="ExternalInput")
    d["wrqT"] = nc.dram_tensor("wrqT", [LAT, HPC * R], BF16,
                               kind="ExternalInput")
    d["wvuT"] = nc.dram_tensor("wvuT", [LAT, HPC * D], BF16,
                               kind="ExternalInput")
    d["woT"] = nc.dram_tensor("woT", [HPC * D, E], BF16,
                              kind="ExternalInput")
    d["bkvd"] = nc.dram_tensor("bkvd", [128, NL], F32, kind="ExternalInput")
    d["bqd"] = nc.dram_tensor("bqd", [128, NL], F32, kind="ExternalInput")
    d["bku"] = nc.dram_tensor("bku", [128, 2], F32, kind="ExternalInput")
    d["bqu"] = nc.dram_tensor("bqu", [128, 2], F32, kind="ExternalInput")
    d["brk"] = nc.dram_tensor("brk", [128, 2], F32, kind="ExternalInput")
    d["brq"] = nc.dram_tensor("brq", [128, 2], F32, kind="ExternalInput")
    d["bvu"] = nc.dram_tensor("bvu", [1, HPC * D], F32, kind="ExternalInput")
    d["onesd"] = nc.dram_tensor("onesd", [128, 1], BF16,
                                kind="ExternalInput")
    d["cosT"] = nc.dram_tensor("cosT", [128, S], BF16, kind="ExternalInput")
    d["sinT"] = nc.dram_tensor("sinT", [128, S], BF16, kind="ExternalInput")
    d["out"] = nc.dram_tensor("out", [S, E], BF16,
                          kind="ExternalOutput")
    if DEBUG_DUMPS:
        d["dbg_sums"] = nc.dram_tensor("dbg_sums", [1, NBLK * QW], F32,
                                       kind="ExternalOutput")
        d["dbg_recip"] = nc.dram_tensor("dbg_recip", [1, NBLK * QW], BF16,
                                        kind="ExternalOutput")
        d["dbg_K"] = nc.dram_tensor("dbg_K", [128, S], BF16,
                                    kind="ExternalOutput")
        d["dbg_Q"] = nc.dram_tensor("dbg_Q", [128, S], BF16,
                                    kind="ExternalOutput")
        d["dbg_V"] = nc.dram_tensor("dbg_V", [128, HPC * D], BF16,
                                    kind="ExternalOutput")
        d["dbg_att"] = nc.dram_tensor("dbg_att", [128, S], BF16,
                                      kind="ExternalOutput")
    return d


def _consts(nc, tc, top, d):
    """Persistent tiles: K/Q/V/att storage, biases, ones, up/out weights."""
    import concourse.mybir as mybir
    F32 = mybir.dt.float32
    BF16 = mybir.dt.bfloat16

    kq_pool = top.enter_context(tc.tile_pool(name="kq", bufs=1))
    v_pool = top.enter_context(tc.tile_pool(name="vp", bufs=1))
    att_pool = top.enter_context(tc.tile_pool(name="att", bufs=1))
    cpool = top.enter_context(tc.tile_pool(name="cp", bufs=1))

    t = {}
    t["K"] = [kq_pool.tile([128, S], BF16, name=f"Kt{h}") for h in range(HPC)]
    t["Q"] = [kq_pool.tile([128, S], BF16, name=f"Qt{h}") for h in range(HPC)]
    t["V"] = [v_pool.tile([128, HPC * D], BF16, name=f"Vt{i}")
              for i in range(NKC)]
    t["att"] = [att_pool.tile([128, S], BF16, name=f"att{h}")
                for h in range(HPC)]

    def ld(name, dram, shape, dt=F32):
        tl = cpool.tile(shape, dt, name=name)
        nc.sync.dma_start(tl[:], dram[:])
        return tl

    t["ones"] = ld("ones_t", d["onesd"], [128, 1], BF16)
    t["bkvd"] = ld("bkvd_t", d["bkvd"], [128, NL])
    t["bqd"] = ld("bqd_t", d["bqd"], [128, NL])
    t["bku"] = ld("bku_t", d["bku"], [128, 2])
    t["bqu"] = ld("bqu_t", d["bqu"], [128, 2])
    t["brk"] = ld("brk_t", d["brk"], [128, 2])
    t["brq"] = ld("brq_t", d["brq"], [128, 2])

    # up-weights + rope tables: tiles allocated here, DMAs emitted inside
    # _emit_A after the first s-chunk's x/weight loads are queued, so the
    # first down-projection matmuls start as early as possible.
    upw = {}
    for nm, w in (("ku", HPC * R), ("qu", HPC * R),
                  ("rq", HPC * R), ("vu", HPC * D)):
        upw[nm] = [cpool.tile([128, w], BF16, name=f"w{nm}{l}")
                   for l in range(NL)]
    t["upw"] = upw
    t["cos"] = cpool.tile([128, S], BF16, name="cos_t")
    t["sin"] = cpool.tile([128, S], BF16, name="sin_t")
    bvu_row = ld("bvu_row", d["bvu"], [1, HPC * D])
    bvu_bc = cpool.tile([128, HPC * D], F32, name="bvu_bc")
    nc.gpsimd.partition_broadcast(bvu_bc[:], bvu_row[:])
    t["bvu_bc"] = bvu_bc

    # Wo streams in late (only needed in phase C)
    wo_t = [cpool.tile([128, E], BF16, name=f"wo{hc}") for hc in range(HPC)]
    t["wo"] = wo_t
    t["wo_dram"] = d["woT"]
    return t


def _phaseA_pools(tc, pa):
    p = {}
    p["xa"] = pa.enter_context(tc.tile_pool(name="xa", bufs=2))
    p["wa"] = pa.enter_context(tc.tile_pool(name="wa", bufs=4))
    p["kvq"] = pa.enter_context(tc.tile_pool(name="kvq", bufs=1))
    p["rp"] = pa.enter_context(tc.tile_pool(name="rp", bufs=2))
    p["psA"] = pa.enter_context(tc.tile_pool(name="psA", bufs=4,
                                             space="PSUM"))
    return p


def _emit_A(nc, tc, d, t, p):
    import concourse.mybir as mybir
    from concourse.alu_op_type import AluOpType
    F32 = mybir.dt.float32
    BF16 = mybir.dt.bfloat16
    F8 = mybir.dt.float8e4
    DR = mybir.MatmulPerfMode.DoubleRow
    K_t, Q_t, V_t, upw = t["K"], t["Q"], t["V"], t["upw"]
    swap_mask = [i ^ 1 for i in range(32)]

    # DMA issue spread over otherwise-idle engine queues for parallelism
    qeng = [nc.sync, nc.gpsimd, nc.scalar]

    for sc in range(NSC):
        ssl = slice(sc * SW, (sc + 1) * SW)
        EPD = 2   # e-chunks per tile/dma
        xts = []
        xts8 = []

        def load_w(wdram, m, dt, nm):
            # weight tile split 3 ways across queues so the stream keeps
            # ahead of the matmul consumer (one m-chunk = ~13.6us of MMs)
            wt = p["wa"].tile([128, NE * 128], dt, name=nm)
            third = NE * 128 // 4
            for j in range(4):
                qeng[(m + j) % 3].dma_start(
                    wt[:, j * third:(j + 1) * third],
                    wdram[m * 128:(m + 1) * 128, j * third:(j + 1) * third])
            return wt

        # the first s-chunk's first weight tile goes ahead of the x
        # chunks in the queues -- PE's first matmul needs both
        wt_first = load_w(d["wkvd_p"], 0, BF16, "wt") if sc == 0 else None

        for i in range(NE // EPD):
            esl = slice(i * EPD * 128, (i + 1) * EPD * 128)
            xi = p["xa"].tile([128, EPD * SW], BF16, name=f"xt{i}")
            qeng[i % 3].dma_start(
                xi[:].rearrange("p (e s) -> p e s", e=EPD),
                d["xT"][esl, ssl].rearrange("(e p) s -> p e s", p=128))
            xts.append(xi)

        def xchunk(e):
            return xts[e // EPD][:, (e % EPD) * SW:(e % EPD + 1) * SW]

        def down_mm(wdram, m, wt=None):
            if wt is None:
                wt = load_w(wdram, m, BF16, "wt")
            ps = p["psA"].tile([128, SW], F32, name="psA_t")
            for e in range(NE):
                nc.tensor.matmul(ps[:], wt[:, e * 128:(e + 1) * 128],
                                 xchunk(e),
                                 start=(e == 0), stop=(e == NE - 1))
            return ps

        def down_mm8(wdram, m):
            # fp8 DoubleRow: each matmul contracts an e-chunk PAIR via
            # 3D APs [p, 2, f]
            wt8 = load_w(wdram, m, F8, "wt8")
            w3 = wt8[:].rearrange("p (e c) -> p e c", e=NE)
            ps = p["psA"].tile([128, SW], F32, name="psA_t")
            for i in range(NE // 2):
                nc.tensor.matmul(
                    ps[:], w3[:, 2 * i:2 * i + 2, :],
                    xts8[i][:].rearrange("p (e s) -> p e s", e=2),
                    start=(i == 0), stop=(i == NE // 2 - 1),
                    perf_mode=DR)
            return ps

        def rope(ps, bias_t, m, dst):
            # ps: [128 rows = 2 heads x 64 rope rows, SW]
            xb = p["rp"].tile([128, SW], BF16, name="xb")
            nc.vector.tensor_scalar_add(xb[:], ps[:], bias_t[:, m:m + 1])
            sh = p["rp"].tile([128, SW], BF16, name="sh")
            nc.vector.stream_shuffle(sh[:], xb[:], swap_mask)
            t1 = p["rp"].tile([128, SW], BF16, name="t1")
            nc.vector.tensor_tensor(t1[:], xb[:], t["cos"][:, ssl],
                                    op=AluOpType.mult)
            t2 = p["rp"].tile([128, SW], BF16, name="t2")
            nc.vector.tensor_tensor(t2[:], sh[:], t["sin"][:, ssl],
                                    op=AluOpType.mult)
            nc.vector.tensor_tensor(dst[2 * m][R:D, ssl], t1[0:R, :],
                                    t2[0:R, :], op=AluOpType.add)
            nc.vector.tensor_tensor(dst[2 * m + 1][R:D, ssl], t1[R:D, :],
                                    t2[R:D, :], op=AluOpType.add)

        def up_mm(src, w, m):
            ps = p["psA"].tile([128, SW], F32, name="psA_t")
            for l in range(NL):
                nc.tensor.matmul(ps[:], w[l][:, m * 128:(m + 1) * 128],
                                 src[l][:], start=(l == 0),
                                 stop=(l == NL - 1))
            return ps

        # latent kv_d down-projection (replicated in batch group)
        kv_s = []
        for m in range(NL):
            ps = down_mm(d["wkvd_p"], m, wt_first if m == 0 else None)
            tl = p["kvq"].tile([128, SW], BF16, name=f"lat{m}")
            nc.scalar.add(tl[:], ps[:], t["bkvd"][:, m:m + 1])
            kv_s.append(tl)
        # x8 chunks for the fp8 q-side: loaded after the kv stream so
        # they don't congest the startup queues (first use is ~40us in)
        for i in range(NE // EPD):
            esl = slice(i * EPD * 128, (i + 1) * EPD * 128)
            x8 = p["xa"].tile([128, EPD * SW], F8, name=f"xt8_{i}")
            qeng[(i + 1) % 3].dma_start(
                x8[:].rearrange("p (e s) -> p e s", e=EPD),
                d["xT8"][esl, ssl].rearrange("(e p) s -> p e s", p=128))
            xts8.append(x8)
        if sc == 0:
            # stream in the up-weights + rope tables behind the first
            # down-projection's loads
            for nm, key in (("ku", "wkuT"), ("qu", "wquT"),
                            ("rq", "wrqT"), ("vu", "wvuT")):
                for l in range(NL):
                    qeng[l % 3].dma_start(
                        t["upw"][nm][l][:],
                        d[key][l * 128:(l + 1) * 128, :])
            nc.scalar.dma_start(t["cos"][:], d["cosT"][:])
            nc.scalar.dma_start(t["sin"][:], d["sinT"][:])
        for m in range(2):  # k1 -> K rows 0..63
            ps = up_mm(kv_s, upw["ku"], m)
            nc.vector.tensor_scalar_add(K_t[2 * m][0:R, ssl], ps[0:R, :],
                                        t["bku"][0:R, m:m + 1])
            nc.vector.tensor_scalar_add(K_t[2 * m + 1][0:R, ssl], ps[R:D, :],
                                        t["bku"][R:D, m:m + 1])
        for j in range(SW // 128):  # V, (s, feat) layout
            ps = p["psA"].tile([128, HPC * D], F32, name="psV_t")
            for l in range(NL):
                nc.tensor.matmul(ps[:], kv_s[l][:, j * 128:(j + 1) * 128],
                                 upw["vu"][l][:], start=(l == 0),
                                 stop=(l == NL - 1))
            nc.vector.tensor_tensor(V_t[sc * (SW // 128) + j][:], ps[:],
                                    t["bvu_bc"][:], op=AluOpType.add)

        # latent q_d down-projection (fp8 DoubleRow, logits-only path)
        q_s = []
        for m in range(NL):
            ps = down_mm8(d["wqd_p8"], m)
            tl = p["kvq"].tile([128, SW], BF16, name=f"latq{m}")
            nc.scalar.add(tl[:], ps[:], t["bqd"][:, m:m + 1])
            q_s.append(tl)
        for m in range(2):  # q1 -> Q rows 0..63
            ps = up_mm(q_s, upw["qu"], m)
            nc.vector.tensor_scalar_add(Q_t[2 * m][0:R, ssl], ps[0:R, :],
                                        t["bqu"][0:R, m:m + 1])
            nc.vector.tensor_scalar_add(Q_t[2 * m + 1][0:R, ssl], ps[R:D, :],
                                        t["bqu"][R:D, m:m + 1])
        for m in range(2):  # rope-q from q_d
            ps = up_mm(q_s, upw["rq"], m)
            rope(ps, t["brq"], m, Q_t)
        # rope-k from x (fp8 DoubleRow, logits-only path)
        for m in range(2):
            ps = down_mm8(d["wrk_p8"], m)
            rope(ps, t["brk"], m, K_t)


def _phaseB_pools(tc, pb):
    p = {}
    p["pe"] = pb.enter_context(tc.tile_pool(name="pe", bufs=6))
    p["sm"] = pb.enter_context(tc.tile_pool(name="sm", bufs=2))
    p["dr"] = pb.enter_context(tc.tile_pool(name="dr", bufs=2,
                                            space="DRAM"))
    p["cb"] = pb.enter_context(tc.tile_pool(name="cb", bufs=2))
    p["psS"] = pb.enter_context(tc.tile_pool(name="psS", bufs=2,
                                             space="PSUM"))
    p["psO"] = pb.enter_context(tc.tile_pool(name="psO", bufs=1,
                                             space="PSUM"))
    p["psR"] = pb.enter_context(tc.tile_pool(name="psR", bufs=1,
                                             space="PSUM"))
    return p


def _emit_B(nc, tc, d, t, p):
    import concourse.mybir as mybir
    from concourse.alu_op_type import AluOpType
    F32 = mybir.dt.float32
    BF16 = mybir.dt.bfloat16
    AF = mybir.ActivationFunctionType
    K_t, Q_t, V_t, att_t = t["K"], t["Q"], t["V"], t["att"]

    GRP = 4   # pe chunks pre-reduced on DVE before each sum matmul
    NCOL = QW // 128

    for h in range(HPC):
        for qp in range(NQB):
            blk = h * NQB + qp
            qa = slice(qp * QW, qp * QW + 512)
            qb = slice(qp * QW + 512, (qp + 1) * QW)
            qsl = slice(qp * QW, (qp + 1) * QW)
            oA = p["psO"].tile([128, 512], F32, name="oA")
            oB = p["psO"].tile([128, 512], F32, name="oB")
            s0 = p["psR"].tile([1, 512], F32, name="s0")
            s1 = p["psR"].tile([1, 512], F32, name="s1")
            pes = {}

            def pv(kk):
                pe = pes.pop(kk)
                nc.tensor.matmul(oA[:], V_t[kk][:, h * D:(h + 1) * D],
                                 pe[:, 0:512], start=(kk == 0),
                                 stop=(kk == NKC - 1))
                nc.tensor.matmul(oB[:], V_t[kk][:, h * D:(h + 1) * D],
                                 pe[:, 512:1024], start=(kk == 0),
                                 stop=(kk == NKC - 1))

            live = {}
            for kk in range(NKC):
                ksl = slice(kk * 128, (kk + 1) * 128)
                pp = p["psS"].tile([128, 1024], F32, name="pp")
                nc.tensor.matmul(pp[:, 0:512], K_t[h][:, ksl], Q_t[h][:, qa],
                                 start=True, stop=True)
                nc.tensor.matmul(pp[:, 512:1024], K_t[h][:, ksl],
                                 Q_t[h][:, qb], start=True, stop=True)
                pe = p["pe"].tile([128, 1024], BF16, name="pet")
                nc.scalar.activation(pe[:], pp[:], AF.Exp, scale=SCALE)
                pes[kk] = pe
                live[kk] = pe
                # 4-way DVE pre-reduction, then one accumulating ones-matmul
                if kk % GRP == GRP - 1:
                    g = kk // GRP
                    a01 = p["pe"].tile([128, 1024], BF16, name="tadd0",
                                       bufs=2)
                    a23 = p["pe"].tile([128, 1024], BF16, name="tadd1",
                                       bufs=2)
                    a03 = p["pe"].tile([128, 1024], BF16, name="tadd2",
                                       bufs=2)
                    with nc.allow_low_precision(
                            reason="4-term bf16 pre-reduction of exp "
                                   "chunks; fp32 PSUM accumulates groups"):
                        nc.vector.tensor_tensor(a01[:], live[kk - 3][:],
                                                live[kk - 2][:],
                                                op=AluOpType.add)
                        nc.vector.tensor_tensor(a23[:], live[kk - 1][:],
                                                live[kk][:],
                                                op=AluOpType.add)
                        nc.vector.tensor_tensor(a03[:], a01[:], a23[:],
                                                op=AluOpType.add)
                    live.clear()
                    nc.tensor.matmul(s0[:], t["ones"][:], a03[:, 0:512],
                                     start=(g == 0),
                                     stop=(g == NKC // GRP - 1))
                    nc.tensor.matmul(s1[:], t["ones"][:], a03[:, 512:1024],
                                     start=(g == 0),
                                     stop=(g == NKC // GRP - 1))
                if kk >= LAG:
                    pv(kk - LAG)
            for kk in range(NKC - LAG, NKC):
                pv(kk)
            # stash unnormalized attention output
            nc.vector.tensor_copy(att_t[h][:, qa], oA[:])
            nc.vector.tensor_copy(att_t[h][:, qb], oB[:])
            # per-block reciprocal pipeline: sums row -> DRAM ->
            # [128, 8] spread -> DVE reciprocal -> DRAM -> row ->
            # broadcast -> normalize.  Overlaps the next block's matmuls.
            srow = p["sm"].tile([1, QW], F32, name="srow")
            nc.vector.tensor_copy(srow[:, 0:512], s0[:])
            nc.vector.tensor_copy(srow[:, 512:QW], s1[:])
            dram_row = p["dr"].tile([1, QW], F32, name="dram_row")
            nc.sync.dma_start(dram_row[:], srow[:])
            rs2 = p["sm"].tile([128, NCOL], F32, name="rs2")
            nc.sync.dma_start(
                rs2[:], dram_row[:].rearrange("o (p c) -> (o p) c", p=128))
            rr2 = p["sm"].tile([128, NCOL], F32, name="rr2")
            nc.vector.reciprocal(rr2[:], rs2[:])
            rr2b = p["sm"].tile([128, NCOL], BF16, name="rr2b")
            nc.vector.tensor_copy(rr2b[:], rr2[:])
            dram_rb = p["dr"].tile([1, QW], BF16, name="dram_rb")
            nc.sync.dma_start(
                dram_rb[:].rearrange("o (p c) -> (o p) c", p=128), rr2b[:])
            recip = p["sm"].tile([1, QW], BF16, name="recip")
            nc.sync.dma_start(recip[:], dram_rb[:])
            cb = p["cb"].tile([128, QW], BF16, name="cbt")
            nc.gpsimd.partition_broadcast(cb[:], recip[:])
            nc.vector.tensor_tensor(att_t[h][:, qsl], att_t[h][:, qsl],
                                    cb[:], op=AluOpType.mult)
            if DEBUG_DUMPS and blk < 2:
                nc.sync.dma_start(
                    d["dbg_sums"][:, blk * QW:(blk + 1) * QW], srow[:])
                nc.sync.dma_start(
                    d["dbg_recip"][:, blk * QW:(blk + 1) * QW], recip[:])
    if DEBUG_DUMPS:
        nc.sync.dma_start(d["dbg_K"][:], t["K"][0][:])
        nc.sync.dma_start(d["dbg_Q"][:], t["Q"][0][:])
        nc.sync.dma_start(d["dbg_V"][:], t["V"][0][:])
        nc.sync.dma_start(d["dbg_att"][:], t["att"][0][:])


def _phaseC_pools(tc, pc):
    p = {}
    p["oc"] = pc.enter_context(tc.tile_pool(name="oc", bufs=4))
    p["psC"] = pc.enter_context(tc.tile_pool(name="psC", bufs=8,
                                             space="PSUM"))
    return p


def _emit_C(nc, tc, d, t, p):
    import concourse.mybir as mybir
    F32 = mybir.dt.float32
    BF16 = mybir.dt.bfloat16
    att_t, wo_t = t["att"], t["wo"]

    qeng = [nc.sync, nc.gpsimd, nc.scalar]
    for hc in range(HPC):
        qeng[hc % 3].dma_start(wo_t[hc][:],
                               t["wo_dram"][hc * 128:(hc + 1) * 128, :])

    for sj in range(S // 128):
        pss = [p["psC"].tile([128, 512], F32, name="psC_t")
               for _ in range(E // 512)]
        for hc in range(HPC):
            for ocn in range(E // 512):
                nc.tensor.matmul(pss[ocn][:],
                                 att_t[hc][:, sj * 128:(sj + 1) * 128],
                                 wo_t[hc][:, ocn * 512:(ocn + 1) * 512],
                                 start=(hc == 0), stop=(hc == HPC - 1))
        for ocn in range(E // 512):
            ob = p["oc"].tile([128, 512], BF16, name="ob")
            if ocn % 2 == 0:
                nc.vector.tensor_copy(ob[:], pss[ocn][:])
            else:
                nc.scalar.copy(ob[:], pss[ocn][:])
            qeng[(sj * 4 + ocn) % 3].dma_start(
                d["out"][sj * 128:(sj + 1) * 128,
                         ocn * 512:(ocn + 1) * 512], ob[:])


def _build_program():
    import concourse.bacc as bacc
    import concourse.tile as tile

    nc = bacc.Bacc("TRN2", target_bir_lowering=False, debug=False,
                   num_devices=NCORES)
    d = _mk(nc)

    with tile.TileContext(nc) as tc, ExitStack() as top:
        t = _consts(nc, tc, top, d)
        with ExitStack() as pa:
            pA = _phaseA_pools(tc, pa)
            _emit_A(nc, tc, d, t, pA)
        with ExitStack() as pb:
            pB = _phaseB_pools(tc, pb)
            _emit_B(nc, tc, d, t, pB)
        with ExitStack() as pc:
            pC = _phaseC_pools(tc, pc)
            _emit_C(nc, tc, d, t, pC)

    nc.compile()
    return nc


def _rope_tables():
    inv_freq = 1.0 / (10000.0 ** (np.arange(0, R, 2, dtype=np.float64) / R))
    t = np.arange(S, dtype=np.float64)
    freqs = np.outer(t, inv_freq)                       # (S, R/2)
    emb = np.concatenate([freqs, freqs], axis=-1)       # (S, R)
    cos = np.cos(emb).astype(np.float32)                # (S, R)
    sin = np.sin(emb).astype(np.float32)
    perm = np.array([(j // 2) if j % 2 == 0 else (j // 2) + R // 2
                     for j in range(R)])
    sign = np.array([-1.0 if j % 2 == 0 else 1.0
                     for j in range(R)], dtype=np.float32)
    cos_p = cos[:, perm].T.copy()                       # (R, S)
    sin_p = (sin[:, perm] * sign[None, :]).T.copy()     # (R, S)
    cosT = np.concatenate([cos_p, cos_p], axis=0)       # (128, S)
    sinT = np.concatenate([sin_p, sin_p], axis=0)
    return cosT, sinT, perm


def _pack_down(Wm, nm, dt=None):
    """Pack [nm*128, E] weight into [m*128+p, e*128+c] layout."""
    import ml_dtypes
    if dt is None:
        dt = ml_dtypes.bfloat16
    a = Wm.reshape(nm, 128, NE, 128)        # [m, c, e, p]
    a = a.transpose(0, 3, 2, 1)             # [m, p, e, c]
    return np.ascontiguousarray(
        a.reshape(nm * 128, NE * 128).astype(dt))


def _per_core_inputs(inputs, core):
    import ml_dtypes
    bf = ml_dtypes.bfloat16
    b, hg = divmod(core, HPC)
    cosT, sinT, perm = _rope_tables()
    hsl64 = np.concatenate([hg * HPC * R + h * R + perm
                            for h in range(HPC)])       # permuted rope rows
    hs64 = slice(hg * HPC * R, (hg + 1) * HPC * R)      # natural 64-rows
    hs128 = slice(hg * HPC * D, (hg + 1) * HPC * D)     # natural 128-rows

    x = np.asarray(inputs["x"], dtype=np.float32)
    f = np.float32
    e4 = ml_dtypes.float8_e4m3
    im = {
        "xT": np.ascontiguousarray(x[b].T.astype(bf)),
        "xT8": np.ascontiguousarray((x[b].T / 8.0).astype(e4)),
        "wkvd_p": _pack_down(np.asarray(inputs["Wkv_d"], f), NL),
        "wqd_p8": _pack_down(8.0 * np.asarray(inputs["Wq_d"], f), NL, e4),
        "wrk_p8": _pack_down(8.0 * np.asarray(inputs["Wrk"], f)[hsl64], 2,
                             e4),
        "wkuT": np.ascontiguousarray(
            np.asarray(inputs["Wk_u"], f)[hs64].T.astype(bf)),
        "wquT": np.ascontiguousarray(
            np.asarray(inputs["Wq_u"], f)[hs64].T.astype(bf)),
        "wrqT": np.ascontiguousarray(
            np.asarray(inputs["Wrq"], f)[hsl64].T.astype(bf)),
        "wvuT": np.ascontiguousarray(
            np.asarray(inputs["Wv_u"], f)[hs128].T.astype(bf)),
        "woT": np.ascontiguousarray(
            np.asarray(inputs["Wo"], f).T[hs128].astype(bf)),
        "bkvd": np.ascontiguousarray(
            np.asarray(inputs["bkv_d"], f).reshape(NL, 128).T),
        "bqd": np.ascontiguousarray(
            np.asarray(inputs["bq_d"], f).reshape(NL, 128).T),
        "bku": np.ascontiguousarray(
            np.asarray(inputs["bk_u"], f)[hs64].reshape(2, 128).T),
        "bqu": np.ascontiguousarray(
            np.asarray(inputs["bq_u"], f)[hs64].reshape(2, 128).T),
        "brk": np.ascontiguousarray(
            np.asarray(inputs["brk"], f)[hsl64].reshape(2, 128).T),
        "brq": np.ascontiguousarray(
            np.asarray(inputs["brq"], f)[hsl64].reshape(2, 128).T),
        "bvu": np.ascontiguousarray(
            np.asarray(inputs["bv_u"], f)[hs128].reshape(1, HPC * D)),
        "onesd": np.ones((128, 1), dtype=bf),
        "cosT": cosT.astype(bf),
        "sinT": sinT.astype(bf),
    }
    return im


def _get_runtime():
    if "rt" in _RT:
        return _RT["rt"]
    import jax
    import numpy as _np
    from jax.sharding import Mesh, PartitionSpec
    from jax.experimental.shard_map import shard_map

    import concourse.mybir as mybir
    from concourse import bass2jax

    nc = _build_program()
    bass2jax.install_neuronx_cc_hook()

    partition_name = (nc.partition_id_tensor.name
                      if nc.partition_id_tensor else None)
    in_names, out_names, out_avals, zero_shapes = [], [], [], []
    for alloc in nc.m.functions[0].allocations:
        if not isinstance(alloc, mybir.MemoryLocationSet):
            continue
        name = alloc.memorylocations[0].name
        if alloc.kind == "ExternalInput":
            if name != partition_name:
                in_names.append(name)
        elif alloc.kind == "ExternalOutput":
            out_names.append(name)
            np_dt = mybir.dt.np(alloc.dtype)
            out_avals.append(jax.core.ShapedArray(
                tuple(alloc.tensor_shape), np_dt))
            zero_shapes.append((tuple(alloc.tensor_shape), np_dt))

    n_params = len(in_names)
    n_outs = len(out_names)
    all_in_names = list(in_names) + list(out_names)
    if partition_name is not None:
        all_in_names.append(partition_name)

    def _body(*args):
        operands = list(args)
        if partition_name is not None:
            operands.append(bass2jax.partition_id_tensor())
        outs = bass2jax._bass_exec_p.bind(
            *operands,
            out_avals=tuple(out_avals),
            in_names=tuple(all_in_names),
            out_names=tuple(out_names),
            lowering_input_output_aliases=(),
            sim_require_finite=True,
            sim_require_nnan=True,
            nc=nc,
        )
        return tuple(outs)

    devices = jax.devices()[:NCORES]
    mesh = Mesh(_np.asarray(devices), ("core",))
    in_specs = (PartitionSpec("core"),) * (n_params + n_outs)
    out_specs = (PartitionSpec("core"),) * n_outs
    donate = tuple(range(n_params, n_params + n_outs))
    sharded = jax.jit(
        shard_map(_body, mesh=mesh, in_specs=in_specs, out_specs=out_specs,
                  check_rep=False),
        donate_argnums=donate, keep_unused=True)

    _RT["rt"] = dict(nc=nc, sharded=sharded, in_names=in_names,
                     out_names=out_names, zero_shapes=zero_shapes,
                     n_outs=n_outs)
    return _RT["rt"]


def _run_cores(in_maps):
    rt = _get_runtime()
    import numpy as _np
    concat_in = [
        _np.concatenate([in_maps[c][name] for c in range(NCORES)], axis=0)
        for name in rt["in_names"]
    ]
    concat_zeros = [
        _np.zeros((NCORES * shp[0],) + shp[1:], dt)
        for (shp, dt) in rt["zero_shapes"]
    ]
    out_arrs = rt["sharded"](*concat_in, *concat_zeros)
    res = []
    for c in range(NCORES):
        m = {}
        for i, name in enumerate(rt["out_names"]):
            shp, dt = rt["zero_shapes"][i]
            m[name] = _np.asarray(out_arrs[i]).reshape((NCORES,) + shp)[c]
        res.append(m)
    return res


def kernel(**inputs):
    in_maps = [_per_core_inputs(inputs, c) for c in range(NCORES)]
    res = _run_cores(in_maps)
    bo = np.asarray(inputs["bo"], dtype=np.float32)
    final = np.empty((B, S, E), dtype=np.float32)
    for b in range(B):
        acc = res[HPC * b]["out"].astype(np.float32)
        for g in range(1, HPC):
            acc = acc + res[HPC * b + g]["out"].astype(np.float32)
        final[b] = acc + bo[None, :]
    return final
